# revision 1
# baseline (speedup 1.0000x reference)
"""Trainium2 Bass kernel for nn_ADLS_13022340842024 (moe_routing).

Data-parallel over batch across 8 NeuronCores (2048 samples/core).

Key algorithmic reductions (host-side, weight/index-only prep):
  * The gated domain-relation matrix Rg is a row-normalized diagonal =>
    h_prime = dom_emb[domain_id] exactly; all hierarchical routing (zeta,
    alpha) is therefore a function of domain_id only -> tiny [D,L]/[D,L,E]
    tables folded with SCALING into per-layer LoRA scale tables [D, E*R].
  * Per-domain towers flattened to one [512,64] matmul + block-diagonal
    [64,8] second layer + one-hot select.

On-device per core:
  * Embedding gather via dma_gather from per-half-batch compacted bf16
    tables (256B rows), PE col-packed transposes -> x_T [feat, batch].
  * 3-layer FCN backbone as feature-major matmuls (bf16 L0/L1, float32r
    L2) with LoRA experts folded into the PSUM accumulation, ACT
    relu+bias epilogues.
  * Towers + one-hot domain select, PE ones-reduce.
"""
import numpy as np
import ml_dtypes
from contextlib import ExitStack

import concourse.bass as bass
import concourse.tile as tile
from concourse import bacc, mybir
from concourse import bass_utils
from concourse.masks import make_identity

BF16 = ml_dtypes.bfloat16

B, F, V, ED = 16384, 32, 100000, 32
NCORES = 8
BL = B // NCORES                 # 2048 samples per core
IN, D0, D1, D2 = 1024, 2048, 1024, 512
D, E, L, R = 8, 8, 3, 4
ER = E * R                       # 32
CH = 256                         # batch chunk per core
NCHUNK = BL // CH                # 8
NIDX = CH * F                    # 8192 gathered rows per chunk
WIDX = NIDX // 16                # 512 idx columns per chunk
NT = 32768                       # compacted table rows (int16-addressable)
EPS, EPS_LN, SCALING = 1e-8, 1e-5, 0.25

_CACHED_NC = None


def _build():
    nc = bacc.Bacc("TRN2", target_bir_lowering=False, debug=False)
    f32, f32r, bf16, i16 = (mybir.dt.float32, mybir.dt.float32r,
                            mybir.dt.bfloat16, mybir.dt.int16)

    tabA = nc.declare_dram_parameter("tabA", [NT, 128], bf16, isOutput=False)
    tabB = nc.declare_dram_parameter("tabB", [NT, 128], bf16, isOutput=False)
    idx_ext = nc.declare_dram_parameter("idx", [128, NCHUNK * WIDX], i16, isOutput=False)
    w0_ext = nc.declare_dram_parameter("w0t", [IN, D0], bf16, isOutput=False)
    w1_ext = nc.declare_dram_parameter("w1t", [D0, D1], bf16, isOutput=False)
    w2_ext = nc.declare_dram_parameter("w2t", [D1, D2], f32r, isOutput=False)
    a0_ext = nc.declare_dram_parameter("a0f", [IN, ER], bf16, isOutput=False)
    a1_ext = nc.declare_dram_parameter("a1f", [D0, ER], bf16, isOutput=False)
    a2_ext = nc.declare_dram_parameter("a2f", [D1, ER], f32r, isOutput=False)
    bm0_ext = nc.declare_dram_parameter("bm0t", [ER, D0], f32r, isOutput=False)
    bm1_ext = nc.declare_dram_parameter("bm1t", [ER, D1], f32r, isOutput=False)
    bm2_ext = nc.declare_dram_parameter("bm2t", [ER, D2], f32r, isOutput=False)
    sc_ext = nc.declare_dram_parameter("scl", [D, 3 * ER], f32r, isOutput=False)
    oh_ext = nc.declare_dram_parameter("onehot", [D, BL], f32r, isOutput=False)
    b0_ext = nc.declare_dram_parameter("b0p", [128, D0 // 128], f32, isOutput=False)
    b1_ext = nc.declare_dram_parameter("b1p", [128, D1 // 128], f32, isOutput=False)
    b2_ext = nc.declare_dram_parameter("b2p", [128, D2 // 128], f32, isOutput=False)
    wt_ext = nc.declare_dram_parameter("wtt", [D2, 64], f32r, isOutput=False)
    bt1_ext = nc.declare_dram_parameter("bt1f", [64, 1], f32, isOutput=False)
    m2_ext = nc.declare_dram_parameter("m2", [64, D], f32r, isOutput=False)
    bt2_ext = nc.declare_dram_parameter("bt2c", [D, 1], f32, isOutput=False)
    ones_ext = nc.declare_dram_parameter("ones8", [D, 1], f32r, isOutput=False)
    out_ext = nc.declare_dram_parameter("out", [1, BL], f32, isOutput=True)

    KT0, KT1, KT2 = IN // 128, D0 // 128, D1 // 128      # 8, 16, 8
    OT0, OT1, OT2 = D0 // 128, D1 // 128, D2 // 128      # 16, 8, 4

    with tile.TileContext(nc) as tc, ExitStack() as ctx:
        wp = ctx.enter_context(tc.tile_pool(name="w", bufs=1))
        gp = ctx.enter_context(tc.tile_pool(name="g", bufs=3))
        xp = ctx.enter_context(tc.tile_pool(name="x", bufs=2))
        hp = ctx.enter_context(tc.tile_pool(name="h", bufs=1))
        sp = ctx.enter_context(tc.tile_pool(name="s", bufs=2))
        pp_tr = ctx.enter_context(tc.tile_pool(name="ptr", bufs=2, space="PSUM"))
        pp_mm = ctx.enter_context(tc.tile_pool(name="pmm", bufs=2, space="PSUM"))
        pp_lo = ctx.enter_context(tc.tile_pool(name="plo", bufs=2, space="PSUM"))
        pp_tw = ctx.enter_context(tc.tile_pool(name="ptw", bufs=2, space="PSUM"))

        ident = wp.tile([128, 128], mybir.dt.bfloat16)
        make_identity(nc, ident[:, :])

        idx_tiles = []
        for c in range(NCHUNK):
            ixt = wp.tile([128, WIDX], mybir.dt.int16, tag=f"idx{c}")
            nc.sync.dma_start(out=ixt[:, :], in_=idx_ext[:, c * WIDX:(c + 1) * WIDX])
            idx_tiles.append(ixt)

        def load_rows(ext, rows, cols, dt, name):
            tiles = []
            for k in range(rows // 128):
                t = wp.tile([128, cols], dt, tag=f"{name}{k}")
                nc.sync.dma_start(out=t[:, :], in_=ext[k * 128:(k + 1) * 128, :])
                tiles.append(t)
            return tiles

        w0t = load_rows(w0_ext, IN, D0, mybir.dt.bfloat16, "w0")
        w1t = load_rows(w1_ext, D0, D1, mybir.dt.bfloat16, "w1")
        w2t = load_rows(w2_ext, D1, D2, mybir.dt.float32r, "w2")
        a0t = load_rows(a0_ext, IN, ER, mybir.dt.bfloat16, "a0")
        a1t = load_rows(a1_ext, D0, ER, mybir.dt.bfloat16, "a1")
        a2t = load_rows(a2_ext, D1, ER, mybir.dt.float32r, "a2")
        wtt = load_rows(wt_ext, D2, 64, mybir.dt.float32r, "wt")

        bm0 = wp.tile([ER, D0], mybir.dt.float32r)
        nc.sync.dma_start(out=bm0[:, :], in_=bm0_ext[:, :])
        bm1 = wp.tile([ER, D1], mybir.dt.float32r)
        nc.sync.dma_start(out=bm1[:, :], in_=bm1_ext[:, :])
        bm2 = wp.tile([ER, D2], mybir.dt.float32r)
        nc.sync.dma_start(out=bm2[:, :], in_=bm2_ext[:, :])
        scl = wp.tile([D, 3 * ER], mybir.dt.float32r)
        nc.sync.dma_start(out=scl[:, :], in_=sc_ext[:, :])
        oh = wp.tile([D, BL], mybir.dt.float32r)
        nc.sync.dma_start(out=oh[:, :], in_=oh_ext[:, :])
        b0p = wp.tile([128, D0 // 128], mybir.dt.float32)
        nc.sync.dma_start(out=b0p[:, :], in_=b0_ext[:, :])
        b1p = wp.tile([128, D1 // 128], mybir.dt.float32)
        nc.sync.dma_start(out=b1p[:, :], in_=b1_ext[:, :])
        b2p = wp.tile([128, D2 // 128], mybir.dt.float32)
        nc.sync.dma_start(out=b2p[:, :], in_=b2_ext[:, :])
        bt1f = wp.tile([64, 1], mybir.dt.float32)
        nc.sync.dma_start(out=bt1f[:, :], in_=bt1_ext[:, :])
        m2 = wp.tile([64, D], mybir.dt.float32r)
        nc.sync.dma_start(out=m2[:, :], in_=m2_ext[:, :])
        bt2c = wp.tile([D, 1], mybir.dt.float32)
        nc.sync.dma_start(out=bt2c[:, :], in_=bt2_ext[:, :])
        ones8 = wp.tile([D, 1], mybir.dt.float32r)
        nc.sync.dma_start(out=ones8[:, :], in_=ones_ext[:, :])

        relu = mybir.ActivationFunctionType.Relu

        def layer(rhs_tile, kt, ot, w_tiles, a_tiles, bm_tile, bias_tile,
                  l_idx, chunk, out_tile, out_dt_bits):
            """One FCN layer on a [128, kt*CH] feature-major rhs."""
            # LoRA A-projection: t = A^T h  -> psum [32, CH]
            ps_t = pp_lo.tile([ER, CH], mybir.dt.float32, tag="lo")
            for k in range(kt):
                nc.tensor.matmul(out=ps_t[:, :], lhsT=a_tiles[k][:, :],
                                 rhs=rhs_tile[:, k * CH:(k + 1) * CH],
                                 start=(k == 0), stop=(k == kt - 1))
            # scale table gathered by domain: s = scl[l].T @ onehot -> [32, CH]
            ps_s = pp_lo.tile([ER, CH], mybir.dt.float32, tag="lo")
            nc.tensor.matmul(out=ps_s[:, :], lhsT=scl[:, l_idx * ER:(l_idx + 1) * ER],
                             rhs=oh[:, chunk * CH:(chunk + 1) * CH],
                             start=True, stop=True)
            s_sb = sp.tile([ER, CH], mybir.dt.float32r, tag="ssb")
            nc.scalar.activation(out=s_sb[:, :], in_=ps_s[:, :],
                                 func=mybir.ActivationFunctionType.Copy)
            t2s = sp.tile([ER, CH], mybir.dt.float32r, tag="t2s")
            nc.vector.tensor_tensor(out=t2s[:, :], in0=ps_t[:, :], in1=s_sb[:, :],
                                    op=mybir.AluOpType.mult)
            for o in range(ot):
                ps = pp_mm.tile([128, CH], mybir.dt.float32, tag="mm")
                for k in range(kt):
                    nc.tensor.matmul(out=ps[:, :],
                                     lhsT=w_tiles[k][:, o * 128:(o + 1) * 128],
                                     rhs=rhs_tile[:, k * CH:(k + 1) * CH],
                                     start=(k == 0), stop=False)
                nc.tensor.matmul(out=ps[:, :], lhsT=bm_tile[:, o * 128:(o + 1) * 128],
                                 rhs=t2s[:, :], start=False, stop=True)
                nc.scalar.activation(out=out_tile[:, o * CH:(o + 1) * CH],
                                     in_=ps[:, :], func=relu,
                                     bias=bias_tile[:, o:o + 1], scale=1.0)

        def backbone(c, xTc):
            h1c = hp.tile([128, OT0 * CH], mybir.dt.bfloat16, tag="h1")
            layer(xTc, KT0, OT0, w0t, a0t, bm0, b0p, 0, c, h1c, 16)
            h2c = hp.tile([128, OT1 * CH], mybir.dt.float32r, tag="h2")
            layer(h1c, KT1, OT1, w1t, a1t, bm1, b1p, 1, c, h2c, 32)
            h3c = hp.tile([128, OT2 * CH], mybir.dt.float32r, tag="h3")
            layer(h2c, KT2, OT2, w2t, a2t, bm2, b2p, 2, c, h3c, 32)

            # towers: t1 = relu(WtT.T @ h3 + bt1f) [64, CH]
            ps_tw = pp_tw.tile([64, CH], mybir.dt.float32, tag="tw")
            for k in range(OT2):
                nc.tensor.matmul(out=ps_tw[:, :], lhsT=wtt[k][:, :],
                                 rhs=h3c[:, k * CH:(k + 1) * CH],
                                 start=(k == 0), stop=(k == OT2 - 1))
            t1s = sp.tile([64, CH], mybir.dt.float32r, tag="t1s")
            nc.scalar.activation(out=t1s[:, :], in_=ps_tw[:, :], func=relu,
                                 bias=bt1f[:, :], scale=1.0)
            # logits_all = M2.T @ t1 + bt2 -> [8, CH]; mask by onehot; reduce
            ps_l = pp_tw.tile([D, CH], mybir.dt.float32, tag="tw")
            nc.tensor.matmul(out=ps_l[:, :], lhsT=m2[:, :], rhs=t1s[:, :],
                             start=True, stop=True)
            lb = sp.tile([D, CH], mybir.dt.float32r, tag="lb")
            nc.vector.tensor_tensor(out=lb[:, :], in0=ps_l[:, :],
                                    in1=bt2c[:, :].to_broadcast([D, CH]),
                                    op=mybir.AluOpType.add)
            mk = sp.tile([D, CH], mybir.dt.float32r, tag="mk")
            nc.vector.tensor_tensor(out=mk[:, :], in0=lb[:, :],
                                    in1=oh[:, c * CH:(c + 1) * CH],
                                    op=mybir.AluOpType.mult)
            ps_f = pp_tw.tile([1, CH], mybir.dt.float32, tag="tw")
            nc.tensor.matmul(out=ps_f[:, :], lhsT=ones8[:, :], rhs=mk[:, :],
                             start=True, stop=True)
            outc = sp.tile([1, CH], mybir.dt.float32, tag="oc")
            nc.vector.tensor_copy(out=outc[:, :], in_=ps_f[:, :])
            nc.sync.dma_start(out=out_ext[0:1, c * CH:(c + 1) * CH], in_=outc[:, :])

        # gathers split in two 4096-row halves for finer G-buffer recycling;
        # x_T assembly copies run on ACT so DVE 2-port bursts never lock the
        # GpSimd SWDGE out of its SBUF descriptor rings.
        for c in range(NCHUNK):
            tab = tabA if c < NCHUNK // 2 else tabB
            xTc = xp.tile([128, 8 * CH], mybir.dt.bfloat16, tag="xT")
            G = gp.tile([128, NIDX // 128, 128], mybir.dt.bfloat16, tag="G")
            nc.gpsimd.dma_gather(
                out_ap=G[:, :, :], in_ap=tab[:, :],
                idxs_ap=idx_tiles[c][:, :],
                num_idxs=NIDX, num_idxs_reg=NIDX, elem_size=128,
                transpose=False, single_packet=False)
            for t in range(8):
                ps = pp_tr.tile([128, 256], mybir.dt.bfloat16, tag="tr")
                for s in range(2):
                    for j in range(4):
                        g = (t * 2 + s) * 4 + j
                        nc.tensor.transpose(
                            out=ps[32 * j:32 * (j + 1), 128 * s:128 * (s + 1)],
                            in_=G[:, g, 0:32], identity=ident[:, :],
                            tile_position=(0, 32 * j))
                nc.vector.tensor_copy(out=xTc[:, t * 256:(t + 1) * 256],
                                      in_=ps[:, :])
            backbone(c, xTc)

    nc.compile()
    return nc


def get_nc():
    global _CACHED_NC
    if _CACHED_NC is None:
        _CACHED_NC = _build()
    return _CACHED_NC


# ---------------- host-side math (exact fp32 mirror of the reference) -------

def _softplus(x):
    return np.logaddexp(0.0, x)


def _ln(x, g, b):
    m = x.mean(-1, keepdims=True)
    v = ((x - m) ** 2).mean(-1, keepdims=True)
    return g * (x - m) / np.sqrt(v + EPS_LN) + b


def _softmax(x):
    e = np.exp(x - x.max(-1, keepdims=True))
    return e / e.sum(-1, keepdims=True)


def _topk_sparse(p, k):
    idx = np.argsort(-p, axis=-1, kind="stable")[..., :k]
    mask = np.zeros_like(p)
    np.put_along_axis(mask, idx, 1.0, axis=-1)
    s = p * mask
    return s / np.maximum(s.sum(-1, keepdims=True), EPS)


def _routing_tables(dom_emb, layer_pos, gate_logits, Wi1, bi1, gi, bti, Wi2,
                    bi2, Wr1, br1, gr, btr, Wr2, br2):
    gate = _softplus(gate_logits.astype(np.float32))
    Rg = np.eye(D, dtype=np.float32) * gate
    Rg = Rg / np.maximum(Rg.sum(1, keepdims=True), EPS)
    hd = Rg @ dom_emb.astype(np.float32)                      # [D, 64]
    ri = np.concatenate([
        np.broadcast_to(hd[:, None, :], (D, L, hd.shape[-1])),
        np.broadcast_to(layer_pos[None].astype(np.float32), (D, L, layer_pos.shape[-1])),
    ], axis=-1)                                               # [D, L, 96]
    hi = np.maximum(_ln(ri @ Wi1.T + bi1, gi, bti), 0.0)
    scores = (hi @ Wi2.T + bi2)[..., 0]
    scores = scores - scores.max(-1, keepdims=True)
    phi = _softmax(scores)
    zeta = _topk_sparse(phi, min(2, L))                       # [D, L]
    hr = np.maximum(_ln(ri @ Wr1.T + br1, gr, btr), 0.0)
    alpha = _topk_sparse(_softmax(hr @ Wr2.T + br2), 2)       # [D, L, E]
    return zeta.astype(np.float32), alpha.astype(np.float32)


def _prep_core(field_idx_c, emb):
    """Compact bf16 gather tables + wrapped int16 device indices for one core."""
    tabs, idx16 = [], []
    half = BL // 2
    for h in range(2):
        fh = field_idx_c[h * half:(h + 1) * half]
        u, inv = np.unique(fh, return_inverse=True)
        tab = np.zeros((NT, 128), dtype=BF16)
        tab[:len(u), 0:ED] = emb[u].astype(BF16)
        tabs.append(tab)
        idx16.append(inv.reshape(fh.shape).astype(np.int16))

    idx_dev = np.zeros((128, NCHUNK * WIDX), dtype=np.int16)
    ch_per_half = NCHUNK // 2
    for c in range(NCHUNK):
        ih = idx16[c // ch_per_half]
        boff = (c % ch_per_half) * CH
        blk3 = ih[boff:boff + CH].reshape(2, 128, 8, 4)       # [s, p, t, j]
        kv = blk3.transpose(2, 0, 3, 1).reshape(NIDX)         # [t, s, j, p]
        blk = kv.reshape(WIDX, 16).T                          # [16, WIDX]
        idx_dev[:, c * WIDX:(c + 1) * WIDX] = np.tile(blk, (8, 1))
    return tabs, idx_dev


def kernel(field_idx, domain_id, emb_table, W0, b0, W1, b1, W2, b2,
           A0, Bm0, A1, Bm1, A2, Bm2, dom_emb, layer_pos, gate_logits,
           Wi1, bi1, gi, bti, Wi2, bi2, Wr1, br1, gr, btr, Wr2, br2,
           Wt1, bt1, Wt2, bt2):
    field_idx = np.asarray(field_idx)
    domain_id = np.asarray(domain_id)
    emb = np.asarray(emb_table, dtype=np.float32)

    zeta, alpha = _routing_tables(
        np.asarray(dom_emb), np.asarray(layer_pos), np.asarray(gate_logits),
        np.asarray(Wi1), np.asarray(bi1), np.asarray(gi), np.asarray(bti),
        np.asarray(Wi2), np.asarray(bi2), np.asarray(Wr1), np.asarray(br1),
        np.asarray(gr), np.asarray(btr), np.asarray(Wr2), np.asarray(br2))

    # per-layer LoRA scale tables packed [D, 3*E*R] (column block per layer)
    scl = np.zeros((D, 3 * ER), dtype=np.float32)
    for l in range(3):
        scl[:, l * ER:(l + 1) * ER] = (
            np.repeat(alpha[:, l, :], R, axis=1) * zeta[:, l, None] * SCALING)

    def prep_w(W):
        return np.ascontiguousarray(np.asarray(W, np.float32).T)

    def prep_a(A, dt):
        return np.ascontiguousarray(
            np.asarray(A, np.float32).transpose(2, 0, 1).reshape(-1, ER)).astype(dt)

    def prep_bm(Bm):
        return np.ascontiguousarray(
            np.asarray(Bm, np.float32).transpose(0, 2, 1).reshape(ER, -1))

    shared = {
        "w0t": prep_w(W0).astype(BF16),
        "w1t": prep_w(W1).astype(BF16),
        "w2t": prep_w(W2),
        "a0f": prep_a(A0, BF16),
        "a1f": prep_a(A1, BF16),
        "a2f": prep_a(A2, np.float32),
        "bm0t": prep_bm(Bm0), "bm1t": prep_bm(Bm1), "bm2t": prep_bm(Bm2),
        "scl": scl,
        "b0p": np.ascontiguousarray(np.asarray(b0, np.float32).reshape(D0 // 128, 128).T),
        "b1p": np.ascontiguousarray(np.asarray(b1, np.float32).reshape(D1 // 128, 128).T),
        "b2p": np.ascontiguousarray(np.asarray(b2, np.float32).reshape(D2 // 128, 128).T),
        "wtt": np.ascontiguousarray(
            np.asarray(Wt1, np.float32).reshape(D * 8, D2).T),
        "bt1f": np.asarray(bt1, np.float32).reshape(64, 1),
        "bt2c": np.asarray(bt2, np.float32).reshape(D, 1),
        "ones8": np.ones((D, 1), np.float32),
    }
    # M2[d*8+o, d'] = Wt2[d, 0, o] iff d == d'
    m2 = np.zeros((64, D), dtype=np.float32)
    wt2 = np.asarray(Wt2, np.float32)
    for d in range(D):
        m2[d * 8:(d + 1) * 8, d] = wt2[d, 0, :]
    shared["m2"] = m2

    in_maps = []
    for ci in range(NCORES):
        sl = slice(ci * BL, (ci + 1) * BL)
        fi = field_idx[sl].astype(np.int64)
        dom = domain_id[sl].astype(np.int64)
        tabs, idx_dev = _prep_core(fi, emb)
        onehot = (dom[None, :] == np.arange(D)[:, None]).astype(np.float32)
        m = dict(shared)
        m.update({"tabA": tabs[0], "tabB": tabs[1], "idx": idx_dev,
                  "onehot": onehot})
        in_maps.append(m)

    nc = get_nc()
    res = bass_utils.run_bass_kernel_spmd(nc, in_maps, core_ids=list(range(NCORES)))
    out = np.concatenate([np.asarray(res.results[i]["out"][0], np.float32)
                          for i in range(NCORES)])
    return out



# revision 3
# speedup vs baseline: 1.6009x; 1.6009x over previous
"""Trainium2 Bass kernel for nn_ADLS_13022340842024 (moe_routing).

Data-parallel over batch across 8 NeuronCores (2048 samples/core).

Key algorithmic reductions (host-side, weight/index-only prep):
  * The gated domain-relation matrix Rg is a row-normalized diagonal =>
    h_prime = dom_emb[domain_id] exactly; all hierarchical routing (zeta,
    alpha) is therefore a function of domain_id only -> tiny [D,L]/[D,L,E]
    tables folded with SCALING into per-layer LoRA scale tables [D, E*R].
  * Embedding lookup done on host: x_T = (emb[field_idx].reshape(B,-1)).T
    shipped pre-transposed in bf16 -> plain contiguous DMA, no gather.
  * Per-domain towers flattened to one [512,64] matmul + block-diagonal
    [64,8] second layer + one-hot select.

On-device per core:
  * 3-layer FCN backbone as feature-major matmuls (bf16 throughout) with
    LoRA experts folded into the PSUM accumulation, ACT relu+bias
    epilogues, 512-wide batch chunks.
  * Towers + one-hot domain select, PE ones-reduce.
"""
import numpy as np
import ml_dtypes
from contextlib import ExitStack

import concourse.bass as bass
import concourse.tile as tile
from concourse import bacc, mybir
from concourse import bass_utils

BF16 = ml_dtypes.bfloat16

B, F, V, ED = 16384, 32, 100000, 32
NCORES = 8
BL = B // NCORES                 # 2048 samples per core
IN, D0, D1, D2 = 1024, 2048, 1024, 512
D, E, L, R = 8, 8, 3, 4
ER = E * R                       # 32
CH = 512                         # batch chunk per core
NCHUNK = BL // CH                # 4
EPS, EPS_LN, SCALING = 1e-8, 1e-5, 0.25

_CACHED_NC = None


def _build():
    nc = bacc.Bacc("TRN2", target_bir_lowering=False, debug=False)
    f32, f32r, bf16 = (mybir.dt.float32, mybir.dt.float32r, mybir.dt.bfloat16)

    xt_ext = nc.declare_dram_parameter("xT", [IN, BL], bf16, isOutput=False)
    w0_ext = nc.declare_dram_parameter("w0t", [IN, D0], bf16, isOutput=False)
    w1_ext = nc.declare_dram_parameter("w1t", [D0, D1], bf16, isOutput=False)
    w2_ext = nc.declare_dram_parameter("w2t", [D1, D2], bf16, isOutput=False)
    a0_ext = nc.declare_dram_parameter("a0f", [IN, ER], bf16, isOutput=False)
    a1_ext = nc.declare_dram_parameter("a1f", [D0, ER], bf16, isOutput=False)
    a2_ext = nc.declare_dram_parameter("a2f", [D1, ER], bf16, isOutput=False)
    bm0_ext = nc.declare_dram_parameter("bm0t", [ER, D0], f32r, isOutput=False)
    bm1_ext = nc.declare_dram_parameter("bm1t", [ER, D1], f32r, isOutput=False)
    bm2_ext = nc.declare_dram_parameter("bm2t", [ER, D2], f32r, isOutput=False)
    sc_ext = nc.declare_dram_parameter("scl", [D, 3 * ER], f32r, isOutput=False)
    oh_ext = nc.declare_dram_parameter("onehot", [D, BL], f32r, isOutput=False)
    b0_ext = nc.declare_dram_parameter("b0p", [128, D0 // 128], f32, isOutput=False)
    b1_ext = nc.declare_dram_parameter("b1p", [128, D1 // 128], f32, isOutput=False)
    b2_ext = nc.declare_dram_parameter("b2p", [128, D2 // 128], f32, isOutput=False)
    wt_ext = nc.declare_dram_parameter("wtt", [D2, 64], bf16, isOutput=False)
    bt1_ext = nc.declare_dram_parameter("bt1f", [64, 1], f32, isOutput=False)
    m2_ext = nc.declare_dram_parameter("m2", [64, D], f32r, isOutput=False)
    bt2_ext = nc.declare_dram_parameter("bt2c", [D, 1], f32, isOutput=False)
    ones_ext = nc.declare_dram_parameter("ones8", [D, 1], f32r, isOutput=False)
    out_ext = nc.declare_dram_parameter("out", [1, BL], f32, isOutput=True)

    KT0, KT1, KT2 = IN // 128, D0 // 128, D1 // 128      # 8, 16, 8
    OT0, OT1, OT2 = D0 // 128, D1 // 128, D2 // 128      # 16, 8, 4

    with tile.TileContext(nc) as tc, ExitStack() as ctx:
        wp = ctx.enter_context(tc.tile_pool(name="w", bufs=1))
        xp = ctx.enter_context(tc.tile_pool(name="x", bufs=2))
        hp = ctx.enter_context(tc.tile_pool(name="h", bufs=1))
        sp = ctx.enter_context(tc.tile_pool(name="s", bufs=2))
        pp_mm = ctx.enter_context(tc.tile_pool(name="pmm", bufs=3, space="PSUM"))
        pp_lo = ctx.enter_context(tc.tile_pool(name="plo", bufs=2, space="PSUM"))
        pp_tw = ctx.enter_context(tc.tile_pool(name="ptw", bufs=2, space="PSUM"))

        # x chunk tiles first on the DMA queue: chunk 0 compute depends on
        # them; weight tiles follow in first-use order.
        def load_rows(ext, rows, cols, dt, name):
            tiles = []
            for k in range(rows // 128):
                t = wp.tile([128, cols], dt, tag=f"{name}{k}")
                nc.sync.dma_start(out=t[:, :], in_=ext[k * 128:(k + 1) * 128, :])
                tiles.append(t)
            return tiles

        xt = []
        for c in range(NCHUNK):
            ctiles = []
            for k in range(KT0):
                t = wp.tile([128, CH], mybir.dt.bfloat16, tag=f"x{c}_{k}")
                nc.sync.dma_start(out=t[:, :],
                                  in_=xt_ext[k * 128:(k + 1) * 128,
                                             c * CH:(c + 1) * CH])
                ctiles.append(t)
            xt.append(ctiles)
            if c == 0:
                w0t = load_rows(w0_ext, IN, D0, mybir.dt.bfloat16, "w0")
                a0t = load_rows(a0_ext, IN, ER, mybir.dt.bfloat16, "a0")

        w1t = load_rows(w1_ext, D0, D1, mybir.dt.bfloat16, "w1")
        w2t = load_rows(w2_ext, D1, D2, mybir.dt.bfloat16, "w2")
        a1t = load_rows(a1_ext, D0, ER, mybir.dt.bfloat16, "a1")
        a2t = load_rows(a2_ext, D1, ER, mybir.dt.bfloat16, "a2")
        wtt = load_rows(wt_ext, D2, 64, mybir.dt.bfloat16, "wt")

        bm0 = wp.tile([ER, D0], mybir.dt.float32r)
        nc.sync.dma_start(out=bm0[:, :], in_=bm0_ext[:, :])
        bm1 = wp.tile([ER, D1], mybir.dt.float32r)
        nc.sync.dma_start(out=bm1[:, :], in_=bm1_ext[:, :])
        bm2 = wp.tile([ER, D2], mybir.dt.float32r)
        nc.sync.dma_start(out=bm2[:, :], in_=bm2_ext[:, :])
        scl = wp.tile([D, 3 * ER], mybir.dt.float32r)
        nc.sync.dma_start(out=scl[:, :], in_=sc_ext[:, :])
        oh = wp.tile([D, BL], mybir.dt.float32r)
        nc.sync.dma_start(out=oh[:, :], in_=oh_ext[:, :])
        b0p = wp.tile([128, D0 // 128], mybir.dt.float32)
        nc.sync.dma_start(out=b0p[:, :], in_=b0_ext[:, :])
        b1p = wp.tile([128, D1 // 128], mybir.dt.float32)
        nc.sync.dma_start(out=b1p[:, :], in_=b1_ext[:, :])
        b2p = wp.tile([128, D2 // 128], mybir.dt.float32)
        nc.sync.dma_start(out=b2p[:, :], in_=b2_ext[:, :])
        bt1f = wp.tile([64, 1], mybir.dt.float32)
        nc.sync.dma_start(out=bt1f[:, :], in_=bt1_ext[:, :])
        m2 = wp.tile([64, D], mybir.dt.float32r)
        nc.sync.dma_start(out=m2[:, :], in_=m2_ext[:, :])
        bt2c = wp.tile([D, 1], mybir.dt.float32)
        nc.sync.dma_start(out=bt2c[:, :], in_=bt2_ext[:, :])
        ones8 = wp.tile([D, 1], mybir.dt.float32r)
        nc.sync.dma_start(out=ones8[:, :], in_=ones_ext[:, :])

        relu = mybir.ActivationFunctionType.Relu

        def layer(rhs_tile, kt, ot, w_tiles, a_tiles, bm_tile, bias_tile,
                  l_idx, chunk, out_tiles):
            """One FCN layer on kt [128, CH] feature-major rhs tiles."""
            # LoRA A-projection: t = A^T h  -> psum [32, CH]
            ps_t = pp_lo.tile([ER, CH], mybir.dt.float32, tag="lo")
            for k in range(kt):
                nc.tensor.matmul(out=ps_t[:, :], lhsT=a_tiles[k][:, :],
                                 rhs=rhs_tile[k][:, :],
                                 start=(k == 0), stop=(k == kt - 1))
            # scale table gathered by domain: s = scl[l].T @ onehot -> [32, CH]
            ps_s = pp_lo.tile([ER, CH], mybir.dt.float32, tag="lo")
            nc.tensor.matmul(out=ps_s[:, :], lhsT=scl[:, l_idx * ER:(l_idx + 1) * ER],
                             rhs=oh[:, chunk * CH:(chunk + 1) * CH],
                             start=True, stop=True)
            s_sb = sp.tile([ER, CH], mybir.dt.float32r, tag="ssb")
            nc.scalar.activation(out=s_sb[:, :], in_=ps_s[:, :],
                                 func=mybir.ActivationFunctionType.Copy)
            t2s = sp.tile([ER, CH], mybir.dt.float32r, tag="t2s")
            nc.vector.tensor_tensor(out=t2s[:, :], in0=ps_t[:, :], in1=s_sb[:, :],
                                    op=mybir.AluOpType.mult)
            for o in range(ot):
                ps = pp_mm.tile([128, CH], mybir.dt.float32, tag="mm")
                for k in range(kt):
                    nc.tensor.matmul(out=ps[:, :],
                                     lhsT=w_tiles[k][:, o * 128:(o + 1) * 128],
                                     rhs=rhs_tile[k][:, :],
                                     start=(k == 0), stop=False)
                nc.tensor.matmul(out=ps[:, :], lhsT=bm_tile[:, o * 128:(o + 1) * 128],
                                 rhs=t2s[:, :], start=False, stop=True)
                nc.scalar.activation(out=out_tiles[o][:, :],
                                     in_=ps[:, :], func=relu,
                                     bias=bias_tile[:, o:o + 1], scale=1.0)

        def backbone(c):
            h1c = [hp.tile([128, CH], mybir.dt.bfloat16, name=f"h1_{o}", tag=f"h1_{o}")
                   for o in range(OT0)]
            layer(xt[c], KT0, OT0, w0t, a0t, bm0, b0p, 0, c, h1c)
            h2c = [hp.tile([128, CH], mybir.dt.bfloat16, name=f"h2_{o}", tag=f"h2_{o}")
                   for o in range(OT1)]
            layer(h1c, KT1, OT1, w1t, a1t, bm1, b1p, 1, c, h2c)
            h3c = [hp.tile([128, CH], mybir.dt.bfloat16, name=f"h3_{o}", tag=f"h3_{o}")
                   for o in range(OT2)]
            layer(h2c, KT2, OT2, w2t, a2t, bm2, b2p, 2, c, h3c)

            # towers: t1 = relu(WtT.T @ h3 + bt1f) [64, CH]
            ps_tw = pp_tw.tile([64, CH], mybir.dt.float32, tag="tw")
            for k in range(OT2):
                nc.tensor.matmul(out=ps_tw[:, :], lhsT=wtt[k][:, :],
                                 rhs=h3c[k][:, :],
                                 start=(k == 0), stop=(k == OT2 - 1))
            t1s = sp.tile([64, CH], mybir.dt.float32r, tag="t1s")
            nc.scalar.activation(out=t1s[:, :], in_=ps_tw[:, :], func=relu,
                                 bias=bt1f[:, :], scale=1.0)
            # logits_all = M2.T @ t1 + bt2 -> [8, CH]; mask by onehot; reduce
            ps_l = pp_tw.tile([D, CH], mybir.dt.float32, tag="tw")
            nc.tensor.matmul(out=ps_l[:, :], lhsT=m2[:, :], rhs=t1s[:, :],
                             start=True, stop=True)
            lb = sp.tile([D, CH], mybir.dt.float32r, tag="lb")
            nc.vector.tensor_tensor(out=lb[:, :], in0=ps_l[:, :],
                                    in1=bt2c[:, :].to_broadcast([D, CH]),
                                    op=mybir.AluOpType.add)
            mk = sp.tile([D, CH], mybir.dt.float32r, tag="mk")
            nc.vector.tensor_tensor(out=mk[:, :], in0=lb[:, :],
                                    in1=oh[:, c * CH:(c + 1) * CH],
                                    op=mybir.AluOpType.mult)
            ps_f = pp_tw.tile([1, CH], mybir.dt.float32, tag="tw")
            nc.tensor.matmul(out=ps_f[:, :], lhsT=ones8[:, :], rhs=mk[:, :],
                             start=True, stop=True)
            outc = sp.tile([1, CH], mybir.dt.float32, tag="oc")
            nc.vector.tensor_copy(out=outc[:, :], in_=ps_f[:, :])
            nc.sync.dma_start(out=out_ext[0:1, c * CH:(c + 1) * CH], in_=outc[:, :])

        for c in range(NCHUNK):
            backbone(c)

    nc.compile()
    return nc


def get_nc():
    global _CACHED_NC
    if _CACHED_NC is None:
        _CACHED_NC = _build()
    return _CACHED_NC


# ---------------- host-side math (exact fp32 mirror of the reference) -------

def _softplus(x):
    return np.logaddexp(0.0, x)


def _ln(x, g, b):
    m = x.mean(-1, keepdims=True)
    v = ((x - m) ** 2).mean(-1, keepdims=True)
    return g * (x - m) / np.sqrt(v + EPS_LN) + b


def _softmax(x):
    e = np.exp(x - x.max(-1, keepdims=True))
    return e / e.sum(-1, keepdims=True)


def _topk_sparse(p, k):
    idx = np.argsort(-p, axis=-1, kind="stable")[..., :k]
    mask = np.zeros_like(p)
    np.put_along_axis(mask, idx, 1.0, axis=-1)
    s = p * mask
    return s / np.maximum(s.sum(-1, keepdims=True), EPS)


def _routing_tables(dom_emb, layer_pos, gate_logits, Wi1, bi1, gi, bti, Wi2,
                    bi2, Wr1, br1, gr, btr, Wr2, br2):
    gate = _softplus(gate_logits.astype(np.float32))
    Rg = np.eye(D, dtype=np.float32) * gate
    Rg = Rg / np.maximum(Rg.sum(1, keepdims=True), EPS)
    hd = Rg @ dom_emb.astype(np.float32)                      # [D, 64]
    ri = np.concatenate([
        np.broadcast_to(hd[:, None, :], (D, L, hd.shape[-1])),
        np.broadcast_to(layer_pos[None].astype(np.float32), (D, L, layer_pos.shape[-1])),
    ], axis=-1)                                               # [D, L, 96]
    hi = np.maximum(_ln(ri @ Wi1.T + bi1, gi, bti), 0.0)
    scores = (hi @ Wi2.T + bi2)[..., 0]
    scores = scores - scores.max(-1, keepdims=True)
    phi = _softmax(scores)
    zeta = _topk_sparse(phi, min(2, L))                       # [D, L]
    hr = np.maximum(_ln(ri @ Wr1.T + br1, gr, btr), 0.0)
    alpha = _topk_sparse(_softmax(hr @ Wr2.T + br2), 2)       # [D, L, E]
    return zeta.astype(np.float32), alpha.astype(np.float32)


def kernel(field_idx, domain_id, emb_table, W0, b0, W1, b1, W2, b2,
           A0, Bm0, A1, Bm1, A2, Bm2, dom_emb, layer_pos, gate_logits,
           Wi1, bi1, gi, bti, Wi2, bi2, Wr1, br1, gr, btr, Wr2, br2,
           Wt1, bt1, Wt2, bt2):
    field_idx = np.asarray(field_idx)
    domain_id = np.asarray(domain_id)
    emb = np.asarray(emb_table, dtype=np.float32)

    zeta, alpha = _routing_tables(
        np.asarray(dom_emb), np.asarray(layer_pos), np.asarray(gate_logits),
        np.asarray(Wi1), np.asarray(bi1), np.asarray(gi), np.asarray(bti),
        np.asarray(Wi2), np.asarray(bi2), np.asarray(Wr1), np.asarray(br1),
        np.asarray(gr), np.asarray(btr), np.asarray(Wr2), np.asarray(br2))

    # per-layer LoRA scale tables packed [D, 3*E*R] (column block per layer)
    scl = np.zeros((D, 3 * ER), dtype=np.float32)
    for l in range(3):
        scl[:, l * ER:(l + 1) * ER] = (
            np.repeat(alpha[:, l, :], R, axis=1) * zeta[:, l, None] * SCALING)

    def prep_w(W, dt=BF16):
        return np.ascontiguousarray(np.asarray(W, np.float32).T).astype(dt)

    def prep_a(A, dt=BF16):
        return np.ascontiguousarray(
            np.asarray(A, np.float32).transpose(2, 0, 1).reshape(-1, ER)).astype(dt)

    def prep_bm(Bm):
        return np.ascontiguousarray(
            np.asarray(Bm, np.float32).transpose(0, 2, 1).reshape(ER, -1))

    shared = {
        "w0t": prep_w(W0),
        "w1t": prep_w(W1),
        "w2t": prep_w(W2),
        "a0f": prep_a(A0),
        "a1f": prep_a(A1),
        "a2f": prep_a(A2),
        "bm0t": prep_bm(Bm0), "bm1t": prep_bm(Bm1), "bm2t": prep_bm(Bm2),
        "scl": scl,
        "b0p": np.ascontiguousarray(np.asarray(b0, np.float32).reshape(D0 // 128, 128).T),
        "b1p": np.ascontiguousarray(np.asarray(b1, np.float32).reshape(D1 // 128, 128).T),
        "b2p": np.ascontiguousarray(np.asarray(b2, np.float32).reshape(D2 // 128, 128).T),
        "wtt": np.ascontiguousarray(
            np.asarray(Wt1, np.float32).reshape(D * 8, D2).T).astype(BF16),
        "bt1f": np.asarray(bt1, np.float32).reshape(64, 1),
        "bt2c": np.asarray(bt2, np.float32).reshape(D, 1),
        "ones8": np.ones((D, 1), np.float32),
    }
    # M2[d*8+o, d'] = Wt2[d, 0, o] iff d == d'
    m2 = np.zeros((64, D), dtype=np.float32)
    wt2 = np.asarray(Wt2, np.float32)
    for d in range(D):
        m2[d * 8:(d + 1) * 8, d] = wt2[d, 0, :]
    shared["m2"] = m2

    # host embedding lookup -> feature-major bf16 [IN, B]
    x = emb[field_idx.astype(np.int64)].reshape(B, IN)
    xT = np.ascontiguousarray(x.T.astype(BF16))

    in_maps = []
    for ci in range(NCORES):
        sl = slice(ci * BL, (ci + 1) * BL)
        dom = domain_id[sl].astype(np.int64)
        onehot = (dom[None, :] == np.arange(D)[:, None]).astype(np.float32)
        m = dict(shared)
        m.update({"xT": np.ascontiguousarray(xT[:, sl]), "onehot": onehot})
        in_maps.append(m)

    nc = get_nc()
    res = bass_utils.run_bass_kernel_spmd(nc, in_maps, core_ids=list(range(NCORES)))
    out = np.concatenate([np.asarray(res.results[i]["out"][0], np.float32)
                          for i in range(NCORES)])
    return out


# revision 6
# speedup vs baseline: 1.7726x; 1.1072x over previous
"""Trainium2 Bass kernel for nn_ADLS_13022340842024 (moe_routing).

Data-parallel over batch across 8 NeuronCores (2048 samples/core).

Key algorithmic reductions (host-side, weight/index-only prep):
  * The gated domain-relation matrix Rg is a row-normalized diagonal =>
    h_prime = dom_emb[domain_id] exactly; all hierarchical routing (zeta,
    alpha) is therefore a function of domain_id only -> per-sample LoRA
    scale rows sclb[96, B] (3 layers x E*R) gathered on host.
  * Embedding lookup done on host: x_T = (emb[field_idx].reshape(B,-1)).T
    shipped pre-transposed in bf16 -> plain contiguous DMA, no gather.
  * Per-domain towers flattened to one [512,64] matmul + block-diagonal
    [64,8] second layer + one-hot select.

On-device per core:
  * 3-layer FCN backbone as feature-major bf16 matmuls; batch processed
    as 2 superchunks of 2x512 so each weight tile is loaded once per two
    512-wide matmuls (halves LDWEIGHTS pressure, longer PE chains).
  * LoRA experts folded into the PSUM accumulation (A-projection +
    per-sample scale + Bm fold), ACT relu+bias epilogues.
  * Towers + one-hot domain select, PE ones-reduce.
"""
import numpy as np
import ml_dtypes
from contextlib import ExitStack

import concourse.bass as bass
import concourse.tile as tile
from concourse import bacc, mybir
from concourse import bass_utils

BF16 = ml_dtypes.bfloat16

B, F, V, ED = 16384, 32, 100000, 32
NCORES = 8
BL = B // NCORES                 # 2048 samples per core
IN, D0, D1, D2 = 1024, 2048, 1024, 512
D, E, L, R = 8, 8, 3, 4
ER = E * R                       # 32
CH = 512                         # batch chunk per core
NCHUNK = BL // CH                # 4
NSUP = NCHUNK // 2               # 2 superchunks of paired chunks
EPS, EPS_LN, SCALING = 1e-8, 1e-5, 0.25

_CACHED_NC = None


def _build():
    nc = bacc.Bacc("TRN2", target_bir_lowering=False, debug=False)
    f32, f32r, bf16 = (mybir.dt.float32, mybir.dt.float32r, mybir.dt.bfloat16)

    xt_ext = nc.declare_dram_parameter("xT", [IN, BL], bf16, isOutput=False)
    w0_ext = nc.declare_dram_parameter("w0t", [IN, D0], bf16, isOutput=False)
    w1_ext = nc.declare_dram_parameter("w1t", [D0, D1], bf16, isOutput=False)
    w2_ext = nc.declare_dram_parameter("w2t", [D1, D2], bf16, isOutput=False)
    a0_ext = nc.declare_dram_parameter("a0f", [IN, ER], bf16, isOutput=False)
    a1_ext = nc.declare_dram_parameter("a1f", [D0, ER], bf16, isOutput=False)
    a2_ext = nc.declare_dram_parameter("a2f", [D1, ER], bf16, isOutput=False)
    bm0_ext = nc.declare_dram_parameter("bm0t", [ER, D0], bf16, isOutput=False)
    bm1_ext = nc.declare_dram_parameter("bm1t", [ER, D1], bf16, isOutput=False)
    bm2_ext = nc.declare_dram_parameter("bm2t", [ER, D2], bf16, isOutput=False)
    sclb_ext = nc.declare_dram_parameter("sclb", [3 * ER, BL], bf16, isOutput=False)
    oh_ext = nc.declare_dram_parameter("onehot", [D, BL], bf16, isOutput=False)
    b0_ext = nc.declare_dram_parameter("b0p", [128, D0 // 128], f32, isOutput=False)
    b1_ext = nc.declare_dram_parameter("b1p", [128, D1 // 128], f32, isOutput=False)
    b2_ext = nc.declare_dram_parameter("b2p", [128, D2 // 128], f32, isOutput=False)
    wt_ext = nc.declare_dram_parameter("wtt", [D2, 64], bf16, isOutput=False)
    bt1_ext = nc.declare_dram_parameter("bt1f", [64, 1], f32, isOutput=False)
    m2_ext = nc.declare_dram_parameter("m2", [64, D], f32r, isOutput=False)
    bt2_ext = nc.declare_dram_parameter("bt2c", [D, 1], f32, isOutput=False)
    ones_ext = nc.declare_dram_parameter("ones8", [D, 1], f32r, isOutput=False)
    out_ext = nc.declare_dram_parameter("out", [1, BL], f32, isOutput=True)

    KT0, KT1, KT2 = IN // 128, D0 // 128, D1 // 128      # 8, 16, 8
    OT0, OT1, OT2 = D0 // 128, D1 // 128, D2 // 128      # 16, 8, 4

    with tile.TileContext(nc) as tc, ExitStack() as ctx:
        wp = ctx.enter_context(tc.tile_pool(name="w", bufs=1))
        hp = ctx.enter_context(tc.tile_pool(name="h", bufs=1))
        sp = ctx.enter_context(tc.tile_pool(name="s", bufs=2))
        pp_mm = ctx.enter_context(tc.tile_pool(name="pmm", bufs=4, space="PSUM"))
        pp_lo = ctx.enter_context(tc.tile_pool(name="plo", bufs=2, space="PSUM"))
        pp_tw = ctx.enter_context(tc.tile_pool(name="ptw", bufs=2, space="PSUM"))

        def load_rows(ext, rows, cols, dt, name, eng):
            tiles = []
            for k in range(rows // 128):
                t = wp.tile([128, cols], dt, tag=f"{name}{k}", name=f"{name}{k}")
                eng.dma_start(out=t[:, :], in_=ext[k * 128:(k + 1) * 128, :])
                tiles.append(t)
            return tiles

        # qSP: the big weight matrices, in use order.
        w0t = load_rows(w0_ext, IN, D0, bf16, "w0", nc.sync)
        w1t = load_rows(w1_ext, D0, D1, bf16, "w1", nc.sync)
        w2t = load_rows(w2_ext, D1, D2, bf16, "w2", nc.sync)
        wtt = load_rows(wt_ext, D2, 64, bf16, "wt", nc.sync)

        # qAct: x chunks + LoRA/aux tensors, in use order.
        xt = []
        for c in range(NCHUNK):
            ctiles = []
            for k in range(KT0):
                t = wp.tile([128, CH], bf16, tag=f"x{c}_{k}", name=f"x{c}_{k}")
                nc.scalar.dma_start(out=t[:, :],
                                    in_=xt_ext[k * 128:(k + 1) * 128,
                                               c * CH:(c + 1) * CH])
                ctiles.append(t)
            xt.append(ctiles)
            if c == 1:
                a0t = load_rows(a0_ext, IN, ER, bf16, "a0", nc.scalar)
                sclb = wp.tile([3 * ER, BL], bf16)
                nc.scalar.dma_start(out=sclb[:, :], in_=sclb_ext[:, :])
                bm0 = wp.tile([ER, D0], bf16)
                nc.scalar.dma_start(out=bm0[:, :], in_=bm0_ext[:, :])
                b0p = wp.tile([128, D0 // 128], f32)
                nc.scalar.dma_start(out=b0p[:, :], in_=b0_ext[:, :])
                a1t = load_rows(a1_ext, D0, ER, bf16, "a1", nc.scalar)
                bm1 = wp.tile([ER, D1], bf16)
                nc.scalar.dma_start(out=bm1[:, :], in_=bm1_ext[:, :])
                b1p = wp.tile([128, D1 // 128], f32)
                nc.scalar.dma_start(out=b1p[:, :], in_=b1_ext[:, :])
                a2t = load_rows(a2_ext, D1, ER, bf16, "a2", nc.scalar)
                bm2 = wp.tile([ER, D2], bf16)
                nc.scalar.dma_start(out=bm2[:, :], in_=bm2_ext[:, :])
                b2p = wp.tile([128, D2 // 128], f32)
                nc.scalar.dma_start(out=b2p[:, :], in_=b2_ext[:, :])
                oh = wp.tile([D, BL], bf16)
                nc.scalar.dma_start(out=oh[:, :], in_=oh_ext[:, :])
                bt1f = wp.tile([64, 1], f32)
                nc.scalar.dma_start(out=bt1f[:, :], in_=bt1_ext[:, :])
                m2 = wp.tile([64, D], f32r)
                nc.scalar.dma_start(out=m2[:, :], in_=m2_ext[:, :])
                bt2c = wp.tile([D, 1], f32)
                nc.scalar.dma_start(out=bt2c[:, :], in_=bt2_ext[:, :])
                ones8 = wp.tile([D, 1], f32r)
                nc.scalar.dma_start(out=ones8[:, :], in_=ones_ext[:, :])

        relu = mybir.ActivationFunctionType.Relu

        def layer(rhs0, rhs1, kt, ot, w_tiles, a_tiles, bm_tile, bias_tile,
                  l_idx, c0, out0, out1):
            """One FCN layer on paired chunks; each weight tile feeds 2 MMs."""
            ps_t0 = pp_lo.tile([ER, CH], f32, tag="lo")
            ps_t1 = pp_lo.tile([ER, CH], f32, tag="lo")
            for k in range(kt):
                nc.tensor.matmul(out=ps_t0[:, :], lhsT=a_tiles[k][:, :],
                                 rhs=rhs0[k][:, :],
                                 start=(k == 0), stop=(k == kt - 1))
                nc.tensor.matmul(out=ps_t1[:, :], lhsT=a_tiles[k][:, :],
                                 rhs=rhs1[k][:, :],
                                 start=(k == 0), stop=(k == kt - 1))
            t2s0 = sp.tile([ER, CH], bf16, tag="t2s0")
            nc.vector.tensor_tensor(
                out=t2s0[:, :], in0=ps_t0[:, :],
                in1=sclb[l_idx * ER:(l_idx + 1) * ER, c0 * CH:(c0 + 1) * CH],
                op=mybir.AluOpType.mult)
            t2s1 = sp.tile([ER, CH], bf16, tag="t2s1")
            nc.vector.tensor_tensor(
                out=t2s1[:, :], in0=ps_t1[:, :],
                in1=sclb[l_idx * ER:(l_idx + 1) * ER,
                         (c0 + 1) * CH:(c0 + 2) * CH],
                op=mybir.AluOpType.mult)
            for o in range(ot):
                ps0 = pp_mm.tile([128, CH], f32, tag="mm")
                ps1 = pp_mm.tile([128, CH], f32, tag="mm")
                for k in range(kt):
                    nc.tensor.matmul(out=ps0[:, :],
                                     lhsT=w_tiles[k][:, o * 128:(o + 1) * 128],
                                     rhs=rhs0[k][:, :],
                                     start=(k == 0), stop=False)
                    nc.tensor.matmul(out=ps1[:, :],
                                     lhsT=w_tiles[k][:, o * 128:(o + 1) * 128],
                                     rhs=rhs1[k][:, :],
                                     start=(k == 0), stop=False)
                nc.tensor.matmul(out=ps0[:, :],
                                 lhsT=bm_tile[:, o * 128:(o + 1) * 128],
                                 rhs=t2s0[:, :], start=False, stop=True)
                nc.tensor.matmul(out=ps1[:, :],
                                 lhsT=bm_tile[:, o * 128:(o + 1) * 128],
                                 rhs=t2s1[:, :], start=False, stop=True)
                nc.scalar.activation(out=out0[o][:, :], in_=ps0[:, :],
                                     func=relu, bias=bias_tile[:, o:o + 1],
                                     scale=1.0)
                nc.scalar.activation(out=out1[o][:, :], in_=ps1[:, :],
                                     func=relu, bias=bias_tile[:, o:o + 1],
                                     scale=1.0)

        def towers(c, h3c):
            ps_tw = pp_tw.tile([64, CH], f32, tag="tw")
            for k in range(OT2):
                nc.tensor.matmul(out=ps_tw[:, :], lhsT=wtt[k][:, :],
                                 rhs=h3c[k][:, :],
                                 start=(k == 0), stop=(k == OT2 - 1))
            t1s = sp.tile([64, CH], f32r, tag="t1s")
            nc.scalar.activation(out=t1s[:, :], in_=ps_tw[:, :], func=relu,
                                 bias=bt1f[:, :], scale=1.0)
            ps_l = pp_tw.tile([D, CH], f32, tag="tw")
            nc.tensor.matmul(out=ps_l[:, :], lhsT=m2[:, :], rhs=t1s[:, :],
                             start=True, stop=True)
            lb = sp.tile([D, CH], f32r, tag="lb")
            nc.vector.tensor_tensor(out=lb[:, :], in0=ps_l[:, :],
                                    in1=bt2c[:, :].to_broadcast([D, CH]),
                                    op=mybir.AluOpType.add)
            mk = sp.tile([D, CH], f32r, tag="mk")
            nc.vector.tensor_tensor(out=mk[:, :], in0=lb[:, :],
                                    in1=oh[:, c * CH:(c + 1) * CH],
                                    op=mybir.AluOpType.mult)
            ps_f = pp_tw.tile([1, CH], f32, tag="tw")
            nc.tensor.matmul(out=ps_f[:, :], lhsT=ones8[:, :], rhs=mk[:, :],
                             start=True, stop=True)
            outc = sp.tile([1, CH], f32, tag="oc")
            nc.vector.tensor_copy(out=outc[:, :], in_=ps_f[:, :])
            nc.sync.dma_start(out=out_ext[0:1, c * CH:(c + 1) * CH],
                              in_=outc[:, :])

        for s in range(NSUP):
            c0 = 2 * s
            h1a = [hp.tile([128, CH], bf16, name=f"h1a_{o}", tag=f"h1a_{o}")
                   for o in range(OT0)]
            h1b = [hp.tile([128, CH], bf16, name=f"h1b_{o}", tag=f"h1b_{o}")
                   for o in range(OT0)]
            layer(xt[c0], xt[c0 + 1], KT0, OT0, w0t, a0t, bm0, b0p, 0, c0,
                  h1a, h1b)
            h2a = [hp.tile([128, CH], bf16, name=f"h2a_{o}", tag=f"h2a_{o}")
                   for o in range(OT1)]
            h2b = [hp.tile([128, CH], bf16, name=f"h2b_{o}", tag=f"h2b_{o}")
                   for o in range(OT1)]
            layer(h1a, h1b, KT1, OT1, w1t, a1t, bm1, b1p, 1, c0, h2a, h2b)
            h3a = [hp.tile([128, CH], bf16, name=f"h3a_{o}", tag=f"h3a_{o}")
                   for o in range(OT2)]
            h3b = [hp.tile([128, CH], bf16, name=f"h3b_{o}", tag=f"h3b_{o}")
                   for o in range(OT2)]
            layer(h2a, h2b, KT2, OT2, w2t, a2t, bm2, b2p, 2, c0, h3a, h3b)
            towers(c0, h3a)
            towers(c0 + 1, h3b)

    nc.compile()
    return nc


def get_nc():
    global _CACHED_NC
    if _CACHED_NC is None:
        _CACHED_NC = _build()
    return _CACHED_NC


# ---------------- host-side math (exact fp32 mirror of the reference) -------

def _softplus(x):
    return np.logaddexp(0.0, x)


def _ln(x, g, b):
    m = x.mean(-1, keepdims=True)
    v = ((x - m) ** 2).mean(-1, keepdims=True)
    return g * (x - m) / np.sqrt(v + EPS_LN) + b


def _softmax(x):
    e = np.exp(x - x.max(-1, keepdims=True))
    return e / e.sum(-1, keepdims=True)


def _topk_sparse(p, k):
    idx = np.argsort(-p, axis=-1, kind="stable")[..., :k]
    mask = np.zeros_like(p)
    np.put_along_axis(mask, idx, 1.0, axis=-1)
    s = p * mask
    return s / np.maximum(s.sum(-1, keepdims=True), EPS)


def _routing_tables(dom_emb, layer_pos, gate_logits, Wi1, bi1, gi, bti, Wi2,
                    bi2, Wr1, br1, gr, btr, Wr2, br2):
    gate = _softplus(gate_logits.astype(np.float32))
    Rg = np.eye(D, dtype=np.float32) * gate
    Rg = Rg / np.maximum(Rg.sum(1, keepdims=True), EPS)
    hd = Rg @ dom_emb.astype(np.float32)                      # [D, 64]
    ri = np.concatenate([
        np.broadcast_to(hd[:, None, :], (D, L, hd.shape[-1])),
        np.broadcast_to(layer_pos[None].astype(np.float32), (D, L, layer_pos.shape[-1])),
    ], axis=-1)                                               # [D, L, 96]
    hi = np.maximum(_ln(ri @ Wi1.T + bi1, gi, bti), 0.0)
    scores = (hi @ Wi2.T + bi2)[..., 0]
    scores = scores - scores.max(-1, keepdims=True)
    phi = _softmax(scores)
    zeta = _topk_sparse(phi, min(2, L))                       # [D, L]
    hr = np.maximum(_ln(ri @ Wr1.T + br1, gr, btr), 0.0)
    alpha = _topk_sparse(_softmax(hr @ Wr2.T + br2), 2)       # [D, L, E]
    return zeta.astype(np.float32), alpha.astype(np.float32)


def kernel(field_idx, domain_id, emb_table, W0, b0, W1, b1, W2, b2,
           A0, Bm0, A1, Bm1, A2, Bm2, dom_emb, layer_pos, gate_logits,
           Wi1, bi1, gi, bti, Wi2, bi2, Wr1, br1, gr, btr, Wr2, br2,
           Wt1, bt1, Wt2, bt2):
    field_idx = np.asarray(field_idx)
    domain_id = np.asarray(domain_id)
    emb = np.asarray(emb_table, dtype=np.float32)

    zeta, alpha = _routing_tables(
        np.asarray(dom_emb), np.asarray(layer_pos), np.asarray(gate_logits),
        np.asarray(Wi1), np.asarray(bi1), np.asarray(gi), np.asarray(bti),
        np.asarray(Wi2), np.asarray(bi2), np.asarray(Wr1), np.asarray(br1),
        np.asarray(gr), np.asarray(btr), np.asarray(Wr2), np.asarray(br2))

    # per-layer LoRA scale tables packed [D, 3*E*R] (column block per layer)
    scl = np.zeros((D, 3 * ER), dtype=np.float32)
    for l in range(3):
        scl[:, l * ER:(l + 1) * ER] = (
            np.repeat(alpha[:, l, :], R, axis=1) * zeta[:, l, None] * SCALING)

    def prep_w(W, dt=BF16):
        return np.ascontiguousarray(np.asarray(W, np.float32).T).astype(dt)

    def prep_a(A, dt=BF16):
        return np.ascontiguousarray(
            np.asarray(A, np.float32).transpose(2, 0, 1).reshape(-1, ER)).astype(dt)

    def prep_bm(Bm):
        return np.ascontiguousarray(
            np.asarray(Bm, np.float32).transpose(0, 2, 1).reshape(ER, -1)).astype(BF16)

    shared = {
        "w0t": prep_w(W0),
        "w1t": prep_w(W1),
        "w2t": prep_w(W2),
        "a0f": prep_a(A0),
        "a1f": prep_a(A1),
        "a2f": prep_a(A2),
        "bm0t": prep_bm(Bm0), "bm1t": prep_bm(Bm1), "bm2t": prep_bm(Bm2),
        "b0p": np.ascontiguousarray(np.asarray(b0, np.float32).reshape(D0 // 128, 128).T),
        "b1p": np.ascontiguousarray(np.asarray(b1, np.float32).reshape(D1 // 128, 128).T),
        "b2p": np.ascontiguousarray(np.asarray(b2, np.float32).reshape(D2 // 128, 128).T),
        "wtt": np.ascontiguousarray(
            np.asarray(Wt1, np.float32).reshape(D * 8, D2).T).astype(BF16),
        "bt1f": np.asarray(bt1, np.float32).reshape(64, 1),
        "bt2c": np.asarray(bt2, np.float32).reshape(D, 1),
        "ones8": np.ones((D, 1), np.float32),
    }
    # M2[d*8+o, d'] = Wt2[d, 0, o] iff d == d'
    m2 = np.zeros((64, D), dtype=np.float32)
    wt2 = np.asarray(Wt2, np.float32)
    for d in range(D):
        m2[d * 8:(d + 1) * 8, d] = wt2[d, 0, :]
    shared["m2"] = m2

    # host embedding lookup -> feature-major bf16 [IN, B]
    x = emb[field_idx.astype(np.int64)].reshape(B, IN)
    xT = np.ascontiguousarray(x.T.astype(BF16))

    in_maps = []
    for ci in range(NCORES):
        sl = slice(ci * BL, (ci + 1) * BL)
        dom = domain_id[sl].astype(np.int64)
        onehot = (dom[None, :] == np.arange(D)[:, None]).astype(BF16)
        sclb = np.ascontiguousarray(scl[dom].T).astype(BF16)  # [96, BL]
        m = dict(shared)
        m.update({"xT": np.ascontiguousarray(xT[:, sl]), "onehot": onehot,
                  "sclb": sclb})
        in_maps.append(m)

    nc = get_nc()
    res = bass_utils.run_bass_kernel_spmd(nc, in_maps, core_ids=list(range(NCORES)))
    out = np.concatenate([np.asarray(res.results[i]["out"][0], np.float32)
                          for i in range(NCORES)])
    return out


# revision 10
# speedup vs baseline: 1.9636x; 1.1077x over previous
"""Trainium2 Bass kernel for nn_ADLS_13022340842024 (moe_routing).

Data-parallel over batch across 8 NeuronCores (2048 samples/core).

Key algorithmic reductions (host-side, weight/index-only prep):
  * The gated domain-relation matrix Rg is a row-normalized diagonal =>
    h_prime = dom_emb[domain_id] exactly; all hierarchical routing (zeta,
    alpha) is a function of domain_id only -> per-sample LoRA scale rows
    gathered on host (sclb2, packed per superchunk).
  * Embedding lookup done on host: x shipped pre-transposed, chunk-packed
    bf16 -> one contiguous DMA per chunk, no gather.
  * Per-domain towers flattened to one [512,64] matmul + block-diagonal
    [64,8] second layer + one-hot select.

On-device per core:
  * 3-layer FCN backbone as feature-major bf16 matmuls; batch processed
    as 2 superchunks of 2x512 so each weight tile feeds two 512-wide
    matmuls back-to-back (longer PE chains, fewer stalls).
  * Weights arrive in column-block-major packed layouts: 4 DMAs per
    matrix, first o-blocks land in ~3us so L0 never waits.
  * LoRA: A-projections for the chunk pair run concurrently in PE column
    groups (tile_position), Bm folds run concurrently in PE row strips.
  * Tower first-layer k-chain interleaved into L2 epilogues to kill the
    end-of-kernel tail; one-hot domain select, PE ones-reduce.
"""
import numpy as np
import ml_dtypes
from contextlib import ExitStack

import concourse.bass as bass
import concourse.tile as tile
from concourse import bacc, mybir
from concourse import bass_utils

BF16 = ml_dtypes.bfloat16

B, F, V, ED = 16384, 32, 100000, 32
NCORES = 8
BL = B // NCORES                 # 2048 samples per core
IN, D0, D1, D2 = 1024, 2048, 1024, 512
D, E, L, R = 8, 8, 3, 4
ER = E * R                       # 32
CH = 512                         # batch chunk per core
NCHUNK = BL // CH                # 4
NSUP = NCHUNK // 2               # 2 superchunks of paired chunks
KT0, KT1, KT2 = IN // 128, D0 // 128, D1 // 128          # 8, 16, 8
OT0, OT1, OT2 = D0 // 128, D1 // 128, D2 // 128          # 16, 8, 4
EPS, EPS_LN, SCALING = 1e-8, 1e-5, 0.25

_CACHED_NC = None


def _build():
    nc = bacc.Bacc("TRN2", target_bir_lowering=False, debug=False)
    f32, f32r, bf16 = (mybir.dt.float32, mybir.dt.float32r, mybir.dt.bfloat16)

    xr_ext = nc.declare_dram_parameter("xr", [128, NCHUNK * KT0 * CH], bf16,
                                       isOutput=False)
    w0_ext = nc.declare_dram_parameter("w0c", [128, OT0 * KT0 * 128], bf16,
                                       isOutput=False)
    w1_ext = nc.declare_dram_parameter("w1c", [128, OT1 * KT1 * 128], bf16,
                                       isOutput=False)
    w2_ext = nc.declare_dram_parameter("w2c", [128, OT2 * KT2 * 128], bf16,
                                       isOutput=False)
    a0_ext = nc.declare_dram_parameter("a0p", [128, KT0 * ER], bf16, isOutput=False)
    a1_ext = nc.declare_dram_parameter("a1p", [128, KT1 * ER], bf16, isOutput=False)
    a2_ext = nc.declare_dram_parameter("a2p", [128, KT2 * ER], bf16, isOutput=False)
    bm0_ext = nc.declare_dram_parameter("bm0t", [2 * ER, D0], bf16, isOutput=False)
    bm1_ext = nc.declare_dram_parameter("bm1t", [2 * ER, D1], bf16, isOutput=False)
    bm2_ext = nc.declare_dram_parameter("bm2t", [2 * ER, D2], bf16, isOutput=False)
    sclb_ext = nc.declare_dram_parameter("sclb2", [2 * ER, 3 * NSUP * CH], bf16,
                                         isOutput=False)
    oh_ext = nc.declare_dram_parameter("onehot", [D, BL], bf16, isOutput=False)
    b0_ext = nc.declare_dram_parameter("b0p", [128, OT0], f32, isOutput=False)
    b1_ext = nc.declare_dram_parameter("b1p", [128, OT1], f32, isOutput=False)
    b2_ext = nc.declare_dram_parameter("b2p", [128, OT2], f32, isOutput=False)
    wt_ext = nc.declare_dram_parameter("wtc", [128, OT2 * 64], bf16, isOutput=False)
    bt1_ext = nc.declare_dram_parameter("bt1f", [64, 1], f32, isOutput=False)
    m2_ext = nc.declare_dram_parameter("m2", [64, D], f32r, isOutput=False)
    bt2_ext = nc.declare_dram_parameter("bt2c", [D, 1], f32, isOutput=False)
    ones_ext = nc.declare_dram_parameter("ones8", [D, 1], f32r, isOutput=False)
    out_ext = nc.declare_dram_parameter("out", [1, BL], f32, isOutput=True)

    with tile.TileContext(nc) as tc, ExitStack() as ctx:
        wp = ctx.enter_context(tc.tile_pool(name="w", bufs=1))
        hp = ctx.enter_context(tc.tile_pool(name="h", bufs=1))
        sp = ctx.enter_context(tc.tile_pool(name="s", bufs=2))
        pp_mm = ctx.enter_context(tc.tile_pool(name="pmm", bufs=3, space="PSUM"))
        pp_lo = ctx.enter_context(tc.tile_pool(name="plo", bufs=1, space="PSUM"))
        pp_tw = ctx.enter_context(tc.tile_pool(name="ptw", bufs=1, space="PSUM"))

        def wsplit(ext, total_cols, nsplit, name, eng):
            """Load a packed weight as nsplit column-slice tiles."""
            tiles = []
            cols = total_cols // nsplit
            for i in range(nsplit):
                t = wp.tile([128, cols], bf16, tag=f"{name}{i}", name=f"{name}{i}")
                eng.dma_start(out=t[:, :], in_=ext[:, i * cols:(i + 1) * cols])
                tiles.append(t)
            return tiles

        # qSP: backbone weights, col-block-major, 4 DMAs each, use order.
        w0c = wsplit(w0_ext, OT0 * KT0 * 128, 4, "w0", nc.sync)   # 4 o-blocks ea
        w1c = wsplit(w1_ext, OT1 * KT1 * 128, 4, "w1", nc.sync)   # 2 o-blocks ea
        w2c = wsplit(w2_ext, OT2 * KT2 * 128, 4, "w2", nc.sync)   # 1 o-block ea
        wtc = wsplit(wt_ext, OT2 * 64, 1, "wt", nc.sync)[0]

        OB0, OB1, OB2 = OT0 // 4, OT1 // 4, OT2 // 4  # o-blocks per w tile

        # qAct: x chunks + LoRA/aux tensors, use order.
        def aux(ext, shape, dt, name):
            t = wp.tile(shape, dt, tag=name, name=name)
            nc.scalar.dma_start(out=t[:, :], in_=ext[:, :])
            return t

        xt = []
        XC = KT0 * CH
        for c in range(NCHUNK):
            t = wp.tile([128, XC], bf16, tag=f"x{c}", name=f"x{c}")
            nc.scalar.dma_start(out=t[:, :], in_=xr_ext[:, c * XC:(c + 1) * XC])
            xt.append(t)
            if c == 0:
                a0p = aux(a0_ext, [128, KT0 * ER], bf16, "a0p")
                sclb2 = aux(sclb_ext, [2 * ER, 3 * NSUP * CH], bf16, "sclb2")
                bm0 = aux(bm0_ext, [2 * ER, D0], bf16, "bm0")
                b0p = aux(b0_ext, [128, OT0], f32, "b0p")
            if c == 1:
                a1p = aux(a1_ext, [128, KT1 * ER], bf16, "a1p")
                bm1 = aux(bm1_ext, [2 * ER, D1], bf16, "bm1")
                b1p = aux(b1_ext, [128, OT1], f32, "b1p")
                a2p = aux(a2_ext, [128, KT2 * ER], bf16, "a2p")
                bm2 = aux(bm2_ext, [2 * ER, D2], bf16, "bm2")
                b2p = aux(b2_ext, [128, OT2], f32, "b2p")
            if c == 2:
                oh = aux(oh_ext, [D, BL], bf16, "oh")
                bt1f = aux(bt1_ext, [64, 1], f32, "bt1f")
                m2 = aux(m2_ext, [64, D], f32r, "m2")
                bt2c = aux(bt2_ext, [D, 1], f32, "bt2c")
                ones8 = aux(ones_ext, [D, 1], f32r, "ones8")

        relu = mybir.ActivationFunctionType.Relu

        def layer(rhs0, rhs1, kt, ot, obpt, wc, a_tile, bm_tile, bias_tile,
                  l_idx, sup, out0, out1, tw=None):
            """One FCN layer on the superchunk's chunk pair.

            rhs0/rhs1: fn(k) -> [128, CH] AP. Each weight tile feeds two
            matmuls; A-projection col-packed, Bm folds row-strip-packed.
            tw=(ps_twA, ps_twB) interleaves the tower k-chain per o-tile.
            """
            ps_t = pp_lo.tile([2 * ER, CH], f32, tag="lo")
            for k in range(kt):
                nc.tensor.matmul(out=ps_t[0:ER, :],
                                 lhsT=a_tile[:, k * ER:(k + 1) * ER],
                                 rhs=rhs0(k), start=(k == 0), stop=(k == kt - 1),
                                 tile_position=(0, 0))
                nc.tensor.matmul(out=ps_t[ER:2 * ER, :],
                                 lhsT=a_tile[:, k * ER:(k + 1) * ER],
                                 rhs=rhs1(k), start=(k == 0), stop=(k == kt - 1),
                                 tile_position=(0, ER))
            t2s = sp.tile([2 * ER, CH], bf16, tag="t2s")
            nc.vector.tensor_tensor(
                out=t2s[:, :], in0=ps_t[:, :],
                in1=sclb2[:, (l_idx * NSUP + sup) * CH:
                          (l_idx * NSUP + sup + 1) * CH],
                op=mybir.AluOpType.mult)
            for o in range(ot):
                wt_ = wc[o // obpt]
                off = (o % obpt) * kt * 128
                ps0 = pp_mm.tile([128, CH], f32, tag="mm")
                ps1 = pp_mm.tile([128, CH], f32, tag="mm")
                for k in range(kt):
                    lhsT = wt_[:, off + k * 128:off + (k + 1) * 128]
                    nc.tensor.matmul(out=ps0[:, :], lhsT=lhsT, rhs=rhs0(k),
                                     start=(k == 0), stop=False)
                    nc.tensor.matmul(out=ps1[:, :], lhsT=lhsT, rhs=rhs1(k),
                                     start=(k == 0), stop=False)
                nc.tensor.matmul(out=ps0[:, :],
                                 lhsT=bm_tile[0:ER, o * 128:(o + 1) * 128],
                                 rhs=t2s[0:ER, :], start=False, stop=True,
                                 tile_position=(0, 0))
                nc.tensor.matmul(out=ps1[:, :],
                                 lhsT=bm_tile[ER:2 * ER, o * 128:(o + 1) * 128],
                                 rhs=t2s[ER:2 * ER, :], start=False, stop=True,
                                 tile_position=(ER, 0))
                nc.scalar.activation(out=out0[o][:, :], in_=ps0[:, :],
                                     func=relu, bias=bias_tile[:, o:o + 1],
                                     scale=1.0)
                nc.scalar.activation(out=out1[o][:, :], in_=ps1[:, :],
                                     func=relu, bias=bias_tile[:, o:o + 1],
                                     scale=1.0)
                if tw is not None:
                    nc.tensor.matmul(out=tw[0][:, :],
                                     lhsT=wtc[:, o * 64:(o + 1) * 64],
                                     rhs=out0[o][:, :],
                                     start=(o == 0), stop=(o == ot - 1))
                    nc.tensor.matmul(out=tw[1][:, :],
                                     lhsT=wtc[:, o * 64:(o + 1) * 64],
                                     rhs=out1[o][:, :],
                                     start=(o == 0), stop=(o == ot - 1))

        def tower_tail(c, ps_tw):
            t1s = sp.tile([64, CH], f32r, tag="t1s")
            nc.scalar.activation(out=t1s[:, :], in_=ps_tw[:, :], func=relu,
                                 bias=bt1f[:, :], scale=1.0)
            ps_l = pp_tw.tile([D, CH], f32, tag="twl")
            nc.tensor.matmul(out=ps_l[:, :], lhsT=m2[:, :], rhs=t1s[:, :],
                             start=True, stop=True)
            lb = sp.tile([D, CH], f32r, tag="lb")
            nc.vector.tensor_tensor(out=lb[:, :], in0=ps_l[:, :],
                                    in1=bt2c[:, :].to_broadcast([D, CH]),
                                    op=mybir.AluOpType.add)
            mk = sp.tile([D, CH], f32r, tag="mk")
            nc.vector.tensor_tensor(out=mk[:, :], in0=lb[:, :],
                                    in1=oh[:, c * CH:(c + 1) * CH],
                                    op=mybir.AluOpType.mult)
            ps_f = pp_tw.tile([1, CH], f32, tag="twl")
            nc.tensor.matmul(out=ps_f[:, :], lhsT=ones8[:, :], rhs=mk[:, :],
                             start=True, stop=True)
            outc = sp.tile([1, CH], f32, tag="oc")
            nc.vector.tensor_copy(out=outc[:, :], in_=ps_f[:, :])
            nc.sync.dma_start(out=out_ext[0:1, c * CH:(c + 1) * CH],
                              in_=outc[:, :])

        for s in range(NSUP):
            c0 = 2 * s
            rx0 = lambda k, _t=xt[c0]: _t[:, k * CH:(k + 1) * CH]
            rx1 = lambda k, _t=xt[c0 + 1]: _t[:, k * CH:(k + 1) * CH]
            h1a = [hp.tile([128, CH], bf16, name=f"h1a_{o}", tag=f"h1a_{o}")
                   for o in range(OT0)]
            h1b = [hp.tile([128, CH], bf16, name=f"h1b_{o}", tag=f"h1b_{o}")
                   for o in range(OT0)]
            layer(rx0, rx1, KT0, OT0, OB0, w0c, a0p, bm0, b0p, 0, s, h1a, h1b)
            rh1a = lambda k: h1a[k][:, :]
            rh1b = lambda k: h1b[k][:, :]
            h2a = [hp.tile([128, CH], bf16, name=f"h2a_{o}", tag=f"h2a_{o}")
                   for o in range(OT1)]
            h2b = [hp.tile([128, CH], bf16, name=f"h2b_{o}", tag=f"h2b_{o}")
                   for o in range(OT1)]
            layer(rh1a, rh1b, KT1, OT1, OB1, w1c, a1p, bm1, b1p, 1, s, h2a, h2b)
            rh2a = lambda k: h2a[k][:, :]
            rh2b = lambda k: h2b[k][:, :]
            h3a = [hp.tile([128, CH], bf16, name=f"h3a_{o}", tag=f"h3a_{o}")
                   for o in range(OT2)]
            h3b = [hp.tile([128, CH], bf16, name=f"h3b_{o}", tag=f"h3b_{o}")
                   for o in range(OT2)]
            ps_twA = pp_tw.tile([64, CH], f32, tag="twA")
            ps_twB = pp_tw.tile([64, CH], f32, tag="twB")
            layer(rh2a, rh2b, KT2, OT2, OB2, w2c, a2p, bm2, b2p, 2, s,
                  h3a, h3b, tw=(ps_twA, ps_twB))
            tower_tail(c0, ps_twA)
            tower_tail(c0 + 1, ps_twB)

    nc.compile()
    return nc


def get_nc():
    global _CACHED_NC
    if _CACHED_NC is None:
        _CACHED_NC = _build()
    return _CACHED_NC


# ---------------- host-side math (exact fp32 mirror of the reference) -------

def _softplus(x):
    return np.logaddexp(0.0, x)


def _ln(x, g, b):
    m = x.mean(-1, keepdims=True)
    v = ((x - m) ** 2).mean(-1, keepdims=True)
    return g * (x - m) / np.sqrt(v + EPS_LN) + b


def _softmax(x):
    e = np.exp(x - x.max(-1, keepdims=True))
    return e / e.sum(-1, keepdims=True)


def _topk_sparse(p, k):
    idx = np.argsort(-p, axis=-1, kind="stable")[..., :k]
    mask = np.zeros_like(p)
    np.put_along_axis(mask, idx, 1.0, axis=-1)
    s = p * mask
    return s / np.maximum(s.sum(-1, keepdims=True), EPS)


def _routing_tables(dom_emb, layer_pos, gate_logits, Wi1, bi1, gi, bti, Wi2,
                    bi2, Wr1, br1, gr, btr, Wr2, br2):
    gate = _softplus(gate_logits.astype(np.float32))
    Rg = np.eye(D, dtype=np.float32) * gate
    Rg = Rg / np.maximum(Rg.sum(1, keepdims=True), EPS)
    hd = Rg @ dom_emb.astype(np.float32)                      # [D, 64]
    ri = np.concatenate([
        np.broadcast_to(hd[:, None, :], (D, L, hd.shape[-1])),
        np.broadcast_to(layer_pos[None].astype(np.float32), (D, L, layer_pos.shape[-1])),
    ], axis=-1)                                               # [D, L, 96]
    hi = np.maximum(_ln(ri @ Wi1.T + bi1, gi, bti), 0.0)
    scores = (hi @ Wi2.T + bi2)[..., 0]
    scores = scores - scores.max(-1, keepdims=True)
    phi = _softmax(scores)
    zeta = _topk_sparse(phi, min(2, L))                       # [D, L]
    hr = np.maximum(_ln(ri @ Wr1.T + br1, gr, btr), 0.0)
    alpha = _topk_sparse(_softmax(hr @ Wr2.T + br2), 2)       # [D, L, E]
    return zeta.astype(np.float32), alpha.astype(np.float32)


def _pack_w(W, kt, ot):
    """[out, in] -> [128, ot*kt*128] with (p, o, k, cc) layout (lhsT blocks)."""
    Wr = np.asarray(W, np.float32).reshape(ot, 128, kt, 128)   # [o, cc, k, p]
    return np.ascontiguousarray(
        Wr.transpose(3, 0, 2, 1).reshape(128, ot * kt * 128)).astype(BF16)


def _pack_a(A, kt):
    """[E, R, in] -> [128, kt*ER] with (p, k, r) layout."""
    Af = np.asarray(A, np.float32).transpose(2, 0, 1).reshape(-1, ER)  # [in, ER]
    Ar = Af.reshape(kt, 128, ER)
    return np.ascontiguousarray(
        Ar.transpose(1, 0, 2).reshape(128, kt * ER)).astype(BF16)


def kernel(field_idx, domain_id, emb_table, W0, b0, W1, b1, W2, b2,
           A0, Bm0, A1, Bm1, A2, Bm2, dom_emb, layer_pos, gate_logits,
           Wi1, bi1, gi, bti, Wi2, bi2, Wr1, br1, gr, btr, Wr2, br2,
           Wt1, bt1, Wt2, bt2):
    field_idx = np.asarray(field_idx)
    domain_id = np.asarray(domain_id)
    emb = np.asarray(emb_table, dtype=np.float32)

    zeta, alpha = _routing_tables(
        np.asarray(dom_emb), np.asarray(layer_pos), np.asarray(gate_logits),
        np.asarray(Wi1), np.asarray(bi1), np.asarray(gi), np.asarray(bti),
        np.asarray(Wi2), np.asarray(bi2), np.asarray(Wr1), np.asarray(br1),
        np.asarray(gr), np.asarray(btr), np.asarray(Wr2), np.asarray(br2))

    # per-layer LoRA scale tables packed [D, 3*E*R] (column block per layer)
    scl = np.zeros((D, 3 * ER), dtype=np.float32)
    for l in range(3):
        scl[:, l * ER:(l + 1) * ER] = (
            np.repeat(alpha[:, l, :], R, axis=1) * zeta[:, l, None] * SCALING)

    def prep_bm(Bm):
        bm = np.asarray(Bm, np.float32).transpose(0, 2, 1).reshape(ER, -1)
        return np.ascontiguousarray(np.tile(bm, (2, 1))).astype(BF16)

    shared = {
        "w0c": _pack_w(W0, KT0, OT0),
        "w1c": _pack_w(W1, KT1, OT1),
        "w2c": _pack_w(W2, KT2, OT2),
        "a0p": _pack_a(A0, KT0),
        "a1p": _pack_a(A1, KT1),
        "a2p": _pack_a(A2, KT2),
        "bm0t": prep_bm(Bm0), "bm1t": prep_bm(Bm1), "bm2t": prep_bm(Bm2),
        "b0p": np.ascontiguousarray(np.asarray(b0, np.float32).reshape(OT0, 128).T),
        "b1p": np.ascontiguousarray(np.asarray(b1, np.float32).reshape(OT1, 128).T),
        "b2p": np.ascontiguousarray(np.asarray(b2, np.float32).reshape(OT2, 128).T),
        "bt1f": np.asarray(bt1, np.float32).reshape(64, 1),
        "bt2c": np.asarray(bt2, np.float32).reshape(D, 1),
        "ones8": np.ones((D, 1), np.float32),
    }
    # wtc: [512, 64] lhsT -> (p, k, cc) layout, k over 4 tiles of 128
    wtt = np.asarray(Wt1, np.float32).reshape(D * 8, D2).T       # [512, 64]
    wtr = wtt.reshape(OT2, 128, 64)                              # [k, p, cc]
    shared["wtc"] = np.ascontiguousarray(
        wtr.transpose(1, 0, 2).reshape(128, OT2 * 64)).astype(BF16)

    # M2[d*8+o, d'] = Wt2[d, 0, o] iff d == d'
    m2 = np.zeros((64, D), dtype=np.float32)
    wt2 = np.asarray(Wt2, np.float32)
    for d in range(D):
        m2[d * 8:(d + 1) * 8, d] = wt2[d, 0, :]
    shared["m2"] = m2

    # host embedding lookup -> chunk-packed feature-major bf16
    x = emb[field_idx.astype(np.int64)].reshape(B, IN)
    xT = x.T.astype(BF16)                                        # [IN, B]

    in_maps = []
    for ci in range(NCORES):
        sl = slice(ci * BL, (ci + 1) * BL)
        dom = domain_id[sl].astype(np.int64)
        onehot = (dom[None, :] == np.arange(D)[:, None]).astype(BF16)
        # xr: [128, c, k, j] packed
        xc = xT[:, sl].reshape(KT0, 128, NCHUNK, CH)             # [k, p, c, j]
        xr = np.ascontiguousarray(
            xc.transpose(1, 2, 0, 3).reshape(128, NCHUNK * KT0 * CH))
        # sclb2: [l*64 + ci*32 + r, s*CH + j]
        sr = scl[dom]                                            # [BL, 96]
        sclb2 = np.zeros((2 * ER, 3 * NSUP * CH), dtype=np.float32)
        for s in range(NSUP):
            for half in range(2):
                c = 2 * s + half
                blk = sr[c * CH:(c + 1) * CH]                    # [CH, 96]
                for l in range(3):
                    sclb2[half * ER:(half + 1) * ER,
                          (l * NSUP + s) * CH:(l * NSUP + s + 1) * CH] = \
                        blk[:, l * ER:(l + 1) * ER].T
        m = dict(shared)
        m.update({"xr": xr, "onehot": onehot, "sclb2": sclb2.astype(BF16)})
        in_maps.append(m)

    nc = get_nc()
    res = bass_utils.run_bass_kernel_spmd(nc, in_maps, core_ids=list(range(NCORES)))
    out = np.concatenate([np.asarray(res.results[i]["out"][0], np.float32)
                          for i in range(NCORES)])
    return out


# revision 12
# speedup vs baseline: 2.0389x; 1.0383x over previous
"""Trainium2 Bass kernel for nn_ADLS_13022340842024 (moe_routing).

Data-parallel over batch across 8 NeuronCores (2048 samples/core).

Key algorithmic reductions (host-side, weight/index-only prep):
  * The gated domain-relation matrix Rg is a row-normalized diagonal =>
    h_prime = dom_emb[domain_id] exactly; all hierarchical routing (zeta,
    alpha) is a function of domain_id only -> per-sample LoRA scale rows
    gathered on host (sclb2, packed per superchunk).
  * Embedding lookup done on host: x shipped pre-transposed, chunk-packed
    bf16 -> one contiguous DMA per chunk, no gather.
  * Per-domain towers flattened to one [512,64] matmul + block-diagonal
    [64,8] second layer + one-hot select.

On-device per core:
  * 3-layer FCN backbone as feature-major bf16 matmuls; batch processed
    as 2 superchunks of 2x512 so each weight tile feeds two 512-wide
    matmuls back-to-back (longer PE chains, fewer stalls).
  * Weights arrive in column-block-major packed layouts: 4 DMAs per
    matrix, first o-blocks land in ~3us so L0 never waits.
  * LoRA: A-projections for the chunk pair run concurrently in PE column
    groups (tile_position), Bm folds run concurrently in PE row strips.
  * Tower first-layer k-chain interleaved into L2 epilogues to kill the
    end-of-kernel tail; one-hot domain select, PE ones-reduce.
"""
import numpy as np
import ml_dtypes
from contextlib import ExitStack

import concourse.bass as bass
import concourse.tile as tile
from concourse import bacc, mybir
from concourse import bass_utils

BF16 = ml_dtypes.bfloat16

B, F, V, ED = 16384, 32, 100000, 32
NCORES = 8
BL = B // NCORES                 # 2048 samples per core
IN, D0, D1, D2 = 1024, 2048, 1024, 512
D, E, L, R = 8, 8, 3, 4
ER = E * R                       # 32
CH = 512                         # batch chunk per core
NCHUNK = BL // CH                # 4
NSUP = NCHUNK // 2               # 2 superchunks of paired chunks
KT0, KT1, KT2 = IN // 128, D0 // 128, D1 // 128          # 8, 16, 8
OT0, OT1, OT2 = D0 // 128, D1 // 128, D2 // 128          # 16, 8, 4
EPS, EPS_LN, SCALING = 1e-8, 1e-5, 0.25

_CACHED_NC = None


def _build():
    nc = bacc.Bacc("TRN2", target_bir_lowering=False, debug=False)
    f32, f32r, bf16 = (mybir.dt.float32, mybir.dt.float32r, mybir.dt.bfloat16)

    xr_ext = nc.declare_dram_parameter("xr", [128, NCHUNK * KT0 * CH], bf16,
                                       isOutput=False)
    w0_ext = nc.declare_dram_parameter("w0c", [128, OT0 * KT0 * 128], bf16,
                                       isOutput=False)
    w1_ext = nc.declare_dram_parameter("w1c", [128, OT1 * KT1 * 128], bf16,
                                       isOutput=False)
    w2_ext = nc.declare_dram_parameter("w2c", [128, OT2 * KT2 * 128], bf16,
                                       isOutput=False)
    a0_ext = nc.declare_dram_parameter("a0p", [128, KT0 * ER], bf16, isOutput=False)
    a1_ext = nc.declare_dram_parameter("a1p", [128, KT1 * ER], bf16, isOutput=False)
    a2_ext = nc.declare_dram_parameter("a2p", [128, KT2 * ER], bf16, isOutput=False)
    bm0_ext = nc.declare_dram_parameter("bm0t", [2 * ER, D0], bf16, isOutput=False)
    bm1_ext = nc.declare_dram_parameter("bm1t", [2 * ER, D1], bf16, isOutput=False)
    bm2_ext = nc.declare_dram_parameter("bm2t", [2 * ER, D2], bf16, isOutput=False)
    sclb_ext = nc.declare_dram_parameter("sclb2", [2 * ER, 3 * NSUP * CH], bf16,
                                         isOutput=False)
    oh_ext = nc.declare_dram_parameter("oh2", [2 * D, NSUP * CH], bf16,
                                       isOutput=False)
    b0_ext = nc.declare_dram_parameter("b0p", [128, OT0], f32, isOutput=False)
    b1_ext = nc.declare_dram_parameter("b1p", [128, OT1], f32, isOutput=False)
    b2_ext = nc.declare_dram_parameter("b2p", [128, OT2], f32, isOutput=False)
    wt_ext = nc.declare_dram_parameter("wtc", [128, OT2 * 64], bf16, isOutput=False)
    bt1_ext = nc.declare_dram_parameter("bt1f2", [128, 1], f32, isOutput=False)
    m2_ext = nc.declare_dram_parameter("m2d", [128, 2 * D], f32r, isOutput=False)
    bt2_ext = nc.declare_dram_parameter("bt2d", [2 * D, 1], f32, isOutput=False)
    ones_ext = nc.declare_dram_parameter("ones2", [2 * D, 2], f32r, isOutput=False)
    out_ext = nc.declare_dram_parameter("out", [1, BL], f32, isOutput=True)

    with tile.TileContext(nc) as tc, ExitStack() as ctx:
        wp = ctx.enter_context(tc.tile_pool(name="w", bufs=1))
        hp = ctx.enter_context(tc.tile_pool(name="h", bufs=1))
        sp = ctx.enter_context(tc.tile_pool(name="s", bufs=2))
        pp_mm = ctx.enter_context(tc.tile_pool(name="pmm", bufs=4, space="PSUM"))
        pp_lo = ctx.enter_context(tc.tile_pool(name="plo", bufs=1, space="PSUM"))
        pp_tw = ctx.enter_context(tc.tile_pool(name="ptw", bufs=1, space="PSUM"))

        def wsplit(ext, total_cols, nsplit, name, eng):
            """Load a packed weight as nsplit column-slice tiles."""
            tiles = []
            cols = total_cols // nsplit
            for i in range(nsplit):
                t = wp.tile([128, cols], bf16, tag=f"{name}{i}", name=f"{name}{i}")
                eng.dma_start(out=t[:, :], in_=ext[:, i * cols:(i + 1) * cols])
                tiles.append(t)
            return tiles

        # Weight col-splits alternate between the two HWDGE queues so the
        # early load runs at both queues' aggregate bandwidth; x/aux tensors
        # slot in by first-use time.
        def wtile(ext, total_cols, nsplit, i, name, eng):
            cols = total_cols // nsplit
            t = wp.tile([128, cols], bf16, tag=f"{name}{i}", name=f"{name}{i}")
            eng.dma_start(out=t[:, :], in_=ext[:, i * cols:(i + 1) * cols])
            return t

        def aux(ext, shape, dt, name, eng):
            t = wp.tile(shape, dt, tag=name, name=name)
            eng.dma_start(out=t[:, :], in_=ext[:, :])
            return t

        XC = KT0 * CH
        W0C, W1C, W2C = OT0 * KT0 * 128, OT1 * KT1 * 128, OT2 * KT2 * 128
        OB0, OB1, OB2 = OT0 // 4, OT1 // 4, OT2 // 4  # o-blocks per w tile

        xt = [None] * NCHUNK
        w0c, w1c, w2c = [None] * 4, [None] * 4, [None] * 4

        w0c[0] = wtile(w0_ext, W0C, 4, 0, "w0", nc.sync)
        # qAct stream, first-use order
        t = wp.tile([128, XC], bf16, tag="x0", name="x0")
        nc.scalar.dma_start(out=t[:, :], in_=xr_ext[:, 0:XC])
        xt[0] = t
        a0p = aux(a0_ext, [128, KT0 * ER], bf16, "a0p", nc.scalar)
        sclb2 = aux(sclb_ext, [2 * ER, 3 * NSUP * CH], bf16, "sclb2", nc.scalar)
        bm0 = aux(bm0_ext, [2 * ER, D0], bf16, "bm0", nc.scalar)
        b0p = aux(b0_ext, [128, OT0], f32, "b0p", nc.scalar)
        t = wp.tile([128, XC], bf16, tag="x1", name="x1")
        nc.scalar.dma_start(out=t[:, :], in_=xr_ext[:, XC:2 * XC])
        xt[1] = t
        w0c[2] = wtile(w0_ext, W0C, 4, 2, "w0", nc.sync)
        w0c[1] = wtile(w0_ext, W0C, 4, 1, "w0", nc.scalar)
        w0c[3] = wtile(w0_ext, W0C, 4, 3, "w0", nc.scalar)
        w1c[0] = wtile(w1_ext, W1C, 4, 0, "w1", nc.sync)
        a1p = aux(a1_ext, [128, KT1 * ER], bf16, "a1p", nc.scalar)
        bm1 = aux(bm1_ext, [2 * ER, D1], bf16, "bm1", nc.scalar)
        b1p = aux(b1_ext, [128, OT1], f32, "b1p", nc.scalar)
        w1c[2] = wtile(w1_ext, W1C, 4, 2, "w1", nc.sync)
        w1c[1] = wtile(w1_ext, W1C, 4, 1, "w1", nc.scalar)
        w1c[3] = wtile(w1_ext, W1C, 4, 3, "w1", nc.scalar)
        w2c[0] = wtile(w2_ext, W2C, 4, 0, "w2", nc.sync)
        a2p = aux(a2_ext, [128, KT2 * ER], bf16, "a2p", nc.scalar)
        bm2 = aux(bm2_ext, [2 * ER, D2], bf16, "bm2", nc.scalar)
        b2p = aux(b2_ext, [128, OT2], f32, "b2p", nc.scalar)
        w2c[2] = wtile(w2_ext, W2C, 4, 2, "w2", nc.sync)
        w2c[1] = wtile(w2_ext, W2C, 4, 1, "w2", nc.scalar)
        w2c[3] = wtile(w2_ext, W2C, 4, 3, "w2", nc.scalar)
        wtc = wtile(wt_ext, OT2 * 64, 1, 0, "wt", nc.sync)
        oh2 = aux(oh_ext, [2 * D, NSUP * CH], bf16, "oh2", nc.sync)
        bt1f2 = aux(bt1_ext, [128, 1], f32, "bt1f2", nc.sync)
        m2d = aux(m2_ext, [128, 2 * D], f32r, "m2d", nc.sync)
        bt2d = aux(bt2_ext, [2 * D, 1], f32, "bt2d", nc.sync)
        ones2 = aux(ones_ext, [2 * D, 2], f32r, "ones2", nc.sync)
        t = wp.tile([128, XC], bf16, tag="x2", name="x2")
        nc.scalar.dma_start(out=t[:, :], in_=xr_ext[:, 2 * XC:3 * XC])
        xt[2] = t
        t = wp.tile([128, XC], bf16, tag="x3", name="x3")
        nc.scalar.dma_start(out=t[:, :], in_=xr_ext[:, 3 * XC:4 * XC])
        xt[3] = t

        relu = mybir.ActivationFunctionType.Relu

        def layer(rhs0, rhs1, kt, ot, obpt, wc, a_tile, bm_tile, bias_tile,
                  l_idx, sup, out0, out1, tw=None):
            """One FCN layer on the superchunk's chunk pair.

            rhs0/rhs1: fn(k) -> [128, CH] AP. Each weight tile feeds two
            matmuls; A-projection col-packed, Bm folds row-strip-packed.
            tw=(ps_twA, ps_twB) interleaves the tower k-chain per o-tile.
            """
            ps_t = pp_lo.tile([2 * ER, CH], f32, tag="lo")
            for k in range(kt):
                nc.tensor.matmul(out=ps_t[0:ER, :],
                                 lhsT=a_tile[:, k * ER:(k + 1) * ER],
                                 rhs=rhs0(k), start=(k == 0), stop=(k == kt - 1),
                                 tile_position=(0, 0))
                nc.tensor.matmul(out=ps_t[ER:2 * ER, :],
                                 lhsT=a_tile[:, k * ER:(k + 1) * ER],
                                 rhs=rhs1(k), start=(k == 0), stop=(k == kt - 1),
                                 tile_position=(0, ER))
            t2s = sp.tile([2 * ER, CH], bf16, tag="t2s")
            nc.vector.tensor_tensor(
                out=t2s[:, :], in0=ps_t[:, :],
                in1=sclb2[:, (l_idx * NSUP + sup) * CH:
                          (l_idx * NSUP + sup + 1) * CH],
                op=mybir.AluOpType.mult)
            for o in range(ot):
                wt_ = wc[o // obpt]
                off = (o % obpt) * kt * 128
                ps0 = pp_mm.tile([128, CH], f32, tag="mm")
                ps1 = pp_mm.tile([128, CH], f32, tag="mm")
                for k in range(kt):
                    lhsT = wt_[:, off + k * 128:off + (k + 1) * 128]
                    nc.tensor.matmul(out=ps0[:, :], lhsT=lhsT, rhs=rhs0(k),
                                     start=(k == 0), stop=False)
                    nc.tensor.matmul(out=ps1[:, :], lhsT=lhsT, rhs=rhs1(k),
                                     start=(k == 0), stop=False)
                nc.tensor.matmul(out=ps0[:, :],
                                 lhsT=bm_tile[0:ER, o * 128:(o + 1) * 128],
                                 rhs=t2s[0:ER, :], start=False, stop=True,
                                 tile_position=(0, 0))
                nc.tensor.matmul(out=ps1[:, :],
                                 lhsT=bm_tile[ER:2 * ER, o * 128:(o + 1) * 128],
                                 rhs=t2s[ER:2 * ER, :], start=False, stop=True,
                                 tile_position=(ER, 0))
                nc.scalar.activation(out=out0[o][:, :], in_=ps0[:, :],
                                     func=relu, bias=bias_tile[:, o:o + 1],
                                     scale=1.0)
                nc.scalar.activation(out=out1[o][:, :], in_=ps1[:, :],
                                     func=relu, bias=bias_tile[:, o:o + 1],
                                     scale=1.0)
                if tw is not None:
                    nc.tensor.matmul(out=tw[0:64, :],
                                     lhsT=wtc[:, o * 64:(o + 1) * 64],
                                     rhs=out0[o][:, :],
                                     start=(o == 0), stop=(o == ot - 1),
                                     tile_position=(0, 0))
                    nc.tensor.matmul(out=tw[64:128, :],
                                     lhsT=wtc[:, o * 64:(o + 1) * 64],
                                     rhs=out1[o][:, :],
                                     start=(o == 0), stop=(o == ot - 1),
                                     tile_position=(0, 64))

        def tower_tail(s, ps_tw):
            t1s2 = sp.tile([128, CH], f32r, tag="t1s2")
            nc.scalar.activation(out=t1s2[:, :], in_=ps_tw[:, :], func=relu,
                                 bias=bt1f2[:, :], scale=1.0)
            ps_l = pp_tw.tile([2 * D, CH], f32, tag="twl")
            nc.tensor.matmul(out=ps_l[:, :], lhsT=m2d[:, :], rhs=t1s2[:, :],
                             start=True, stop=True)
            lb = sp.tile([2 * D, CH], f32r, tag="lb")
            nc.vector.tensor_tensor(out=lb[:, :], in0=ps_l[:, :],
                                    in1=bt2d[:, :].to_broadcast([2 * D, CH]),
                                    op=mybir.AluOpType.add)
            mk = sp.tile([2 * D, CH], f32r, tag="mk")
            nc.vector.tensor_tensor(out=mk[:, :], in0=lb[:, :],
                                    in1=oh2[:, s * CH:(s + 1) * CH],
                                    op=mybir.AluOpType.mult)
            ps_f = pp_tw.tile([2, CH], f32, tag="twf")
            nc.tensor.matmul(out=ps_f[:, :], lhsT=ones2[:, :], rhs=mk[:, :],
                             start=True, stop=True)
            outc = sp.tile([2, CH], f32, tag="oc")
            nc.vector.tensor_copy(out=outc[:, :], in_=ps_f[:, :])
            nc.sync.dma_start(out=out_ext[0:1, 2 * s * CH:(2 * s + 2) * CH],
                              in_=outc[:, :])

        for s in range(NSUP):
            c0 = 2 * s
            rx0 = lambda k, _t=xt[c0]: _t[:, k * CH:(k + 1) * CH]
            rx1 = lambda k, _t=xt[c0 + 1]: _t[:, k * CH:(k + 1) * CH]
            h1a = [hp.tile([128, CH], bf16, name=f"h1a_{o}", tag=f"h1a_{o}")
                   for o in range(OT0)]
            h1b = [hp.tile([128, CH], bf16, name=f"h1b_{o}", tag=f"h1b_{o}")
                   for o in range(OT0)]
            layer(rx0, rx1, KT0, OT0, OB0, w0c, a0p, bm0, b0p, 0, s, h1a, h1b)
            rh1a = lambda k: h1a[k][:, :]
            rh1b = lambda k: h1b[k][:, :]
            h2a = [hp.tile([128, CH], bf16, name=f"h2a_{o}", tag=f"h2a_{o}")
                   for o in range(OT1)]
            h2b = [hp.tile([128, CH], bf16, name=f"h2b_{o}", tag=f"h2b_{o}")
                   for o in range(OT1)]
            layer(rh1a, rh1b, KT1, OT1, OB1, w1c, a1p, bm1, b1p, 1, s, h2a, h2b)
            rh2a = lambda k: h2a[k][:, :]
            rh2b = lambda k: h2b[k][:, :]
            h3a = [hp.tile([128, CH], bf16, name=f"h3a_{o}", tag=f"h3a_{o}")
                   for o in range(OT2)]
            h3b = [hp.tile([128, CH], bf16, name=f"h3b_{o}", tag=f"h3b_{o}")
                   for o in range(OT2)]
            ps_tw2 = pp_tw.tile([128, CH], f32, tag="tw2")
            layer(rh2a, rh2b, KT2, OT2, OB2, w2c, a2p, bm2, b2p, 2, s,
                  h3a, h3b, tw=ps_tw2)
            tower_tail(s, ps_tw2)

    nc.compile()
    return nc


def get_nc():
    global _CACHED_NC
    if _CACHED_NC is None:
        _CACHED_NC = _build()
    return _CACHED_NC


# ---------------- host-side math (exact fp32 mirror of the reference) -------

def _softplus(x):
    return np.logaddexp(0.0, x)


def _ln(x, g, b):
    m = x.mean(-1, keepdims=True)
    v = ((x - m) ** 2).mean(-1, keepdims=True)
    return g * (x - m) / np.sqrt(v + EPS_LN) + b


def _softmax(x):
    e = np.exp(x - x.max(-1, keepdims=True))
    return e / e.sum(-1, keepdims=True)


def _topk_sparse(p, k):
    idx = np.argsort(-p, axis=-1, kind="stable")[..., :k]
    mask = np.zeros_like(p)
    np.put_along_axis(mask, idx, 1.0, axis=-1)
    s = p * mask
    return s / np.maximum(s.sum(-1, keepdims=True), EPS)


def _routing_tables(dom_emb, layer_pos, gate_logits, Wi1, bi1, gi, bti, Wi2,
                    bi2, Wr1, br1, gr, btr, Wr2, br2):
    gate = _softplus(gate_logits.astype(np.float32))
    Rg = np.eye(D, dtype=np.float32) * gate
    Rg = Rg / np.maximum(Rg.sum(1, keepdims=True), EPS)
    hd = Rg @ dom_emb.astype(np.float32)                      # [D, 64]
    ri = np.concatenate([
        np.broadcast_to(hd[:, None, :], (D, L, hd.shape[-1])),
        np.broadcast_to(layer_pos[None].astype(np.float32), (D, L, layer_pos.shape[-1])),
    ], axis=-1)                                               # [D, L, 96]
    hi = np.maximum(_ln(ri @ Wi1.T + bi1, gi, bti), 0.0)
    scores = (hi @ Wi2.T + bi2)[..., 0]
    scores = scores - scores.max(-1, keepdims=True)
    phi = _softmax(scores)
    zeta = _topk_sparse(phi, min(2, L))                       # [D, L]
    hr = np.maximum(_ln(ri @ Wr1.T + br1, gr, btr), 0.0)
    alpha = _topk_sparse(_softmax(hr @ Wr2.T + br2), 2)       # [D, L, E]
    return zeta.astype(np.float32), alpha.astype(np.float32)


def _pack_w(W, kt, ot):
    """[out, in] -> [128, ot*kt*128] with (p, o, k, cc) layout (lhsT blocks)."""
    Wr = np.asarray(W, np.float32).reshape(ot, 128, kt, 128)   # [o, cc, k, p]
    return np.ascontiguousarray(
        Wr.transpose(3, 0, 2, 1).reshape(128, ot * kt * 128)).astype(BF16)


def _pack_a(A, kt):
    """[E, R, in] -> [128, kt*ER] with (p, k, r) layout."""
    Af = np.asarray(A, np.float32).transpose(2, 0, 1).reshape(-1, ER)  # [in, ER]
    Ar = Af.reshape(kt, 128, ER)
    return np.ascontiguousarray(
        Ar.transpose(1, 0, 2).reshape(128, kt * ER)).astype(BF16)


def kernel(field_idx, domain_id, emb_table, W0, b0, W1, b1, W2, b2,
           A0, Bm0, A1, Bm1, A2, Bm2, dom_emb, layer_pos, gate_logits,
           Wi1, bi1, gi, bti, Wi2, bi2, Wr1, br1, gr, btr, Wr2, br2,
           Wt1, bt1, Wt2, bt2):
    field_idx = np.asarray(field_idx)
    domain_id = np.asarray(domain_id)
    emb = np.asarray(emb_table, dtype=np.float32)

    zeta, alpha = _routing_tables(
        np.asarray(dom_emb), np.asarray(layer_pos), np.asarray(gate_logits),
        np.asarray(Wi1), np.asarray(bi1), np.asarray(gi), np.asarray(bti),
        np.asarray(Wi2), np.asarray(bi2), np.asarray(Wr1), np.asarray(br1),
        np.asarray(gr), np.asarray(btr), np.asarray(Wr2), np.asarray(br2))

    # per-layer LoRA scale tables packed [D, 3*E*R] (column block per layer)
    scl = np.zeros((D, 3 * ER), dtype=np.float32)
    for l in range(3):
        scl[:, l * ER:(l + 1) * ER] = (
            np.repeat(alpha[:, l, :], R, axis=1) * zeta[:, l, None] * SCALING)

    def prep_bm(Bm):
        bm = np.asarray(Bm, np.float32).transpose(0, 2, 1).reshape(ER, -1)
        return np.ascontiguousarray(np.tile(bm, (2, 1))).astype(BF16)

    shared = {
        "w0c": _pack_w(W0, KT0, OT0),
        "w1c": _pack_w(W1, KT1, OT1),
        "w2c": _pack_w(W2, KT2, OT2),
        "a0p": _pack_a(A0, KT0),
        "a1p": _pack_a(A1, KT1),
        "a2p": _pack_a(A2, KT2),
        "bm0t": prep_bm(Bm0), "bm1t": prep_bm(Bm1), "bm2t": prep_bm(Bm2),
        "b0p": np.ascontiguousarray(np.asarray(b0, np.float32).reshape(OT0, 128).T),
        "b1p": np.ascontiguousarray(np.asarray(b1, np.float32).reshape(OT1, 128).T),
        "b2p": np.ascontiguousarray(np.asarray(b2, np.float32).reshape(OT2, 128).T),
        "bt1f2": np.tile(np.asarray(bt1, np.float32).reshape(64, 1), (2, 1)),
        "bt2d": np.tile(np.asarray(bt2, np.float32).reshape(D, 1), (2, 1)),
    }
    ones2 = np.zeros((2 * D, 2), dtype=np.float32)
    ones2[0:D, 0] = 1.0
    ones2[D:2 * D, 1] = 1.0
    shared["ones2"] = ones2
    # wtc: [512, 64] lhsT -> (p, k, cc) layout, k over 4 tiles of 128
    wtt = np.asarray(Wt1, np.float32).reshape(D * 8, D2).T       # [512, 64]
    wtr = wtt.reshape(OT2, 128, 64)                              # [k, p, cc]
    shared["wtc"] = np.ascontiguousarray(
        wtr.transpose(1, 0, 2).reshape(128, OT2 * 64)).astype(BF16)

    # M2[d*8+o, d'] = Wt2[d, 0, o] iff d == d'; block-diag for the chunk pair
    m2 = np.zeros((64, D), dtype=np.float32)
    wt2 = np.asarray(Wt2, np.float32)
    for d in range(D):
        m2[d * 8:(d + 1) * 8, d] = wt2[d, 0, :]
    m2d = np.zeros((128, 2 * D), dtype=np.float32)
    m2d[0:64, 0:D] = m2
    m2d[64:128, D:2 * D] = m2
    shared["m2d"] = m2d

    # host embedding lookup -> chunk-packed feature-major bf16
    x = emb[field_idx.astype(np.int64)].reshape(B, IN)
    xT = x.T.astype(BF16)                                        # [IN, B]

    in_maps = []
    for ci in range(NCORES):
        sl = slice(ci * BL, (ci + 1) * BL)
        dom = domain_id[sl].astype(np.int64)
        onehot = (dom[None, :] == np.arange(D)[:, None]).astype(np.float32)
        oh2 = np.zeros((2 * D, NSUP * CH), dtype=np.float32)
        for s in range(NSUP):
            oh2[0:D, s * CH:(s + 1) * CH] = onehot[:, 2 * s * CH:(2 * s + 1) * CH]
            oh2[D:2 * D, s * CH:(s + 1) * CH] = \
                onehot[:, (2 * s + 1) * CH:(2 * s + 2) * CH]
        # xr: [128, c, k, j] packed
        xc = xT[:, sl].reshape(KT0, 128, NCHUNK, CH)             # [k, p, c, j]
        xr = np.ascontiguousarray(
            xc.transpose(1, 2, 0, 3).reshape(128, NCHUNK * KT0 * CH))
        # sclb2: [l*64 + ci*32 + r, s*CH + j]
        sr = scl[dom]                                            # [BL, 96]
        sclb2 = np.zeros((2 * ER, 3 * NSUP * CH), dtype=np.float32)
        for s in range(NSUP):
            for half in range(2):
                c = 2 * s + half
                blk = sr[c * CH:(c + 1) * CH]                    # [CH, 96]
                for l in range(3):
                    sclb2[half * ER:(half + 1) * ER,
                          (l * NSUP + s) * CH:(l * NSUP + s + 1) * CH] = \
                        blk[:, l * ER:(l + 1) * ER].T
        m = dict(shared)
        m.update({"xr": xr, "oh2": oh2.astype(BF16),
                  "sclb2": sclb2.astype(BF16)})
        in_maps.append(m)

    nc = get_nc()
    res = bass_utils.run_bass_kernel_spmd(nc, in_maps, core_ids=list(range(NCORES)))
    out = np.concatenate([np.asarray(res.results[i]["out"][0], np.float32)
                          for i in range(NCORES)])
    return out


# revision 13
# speedup vs baseline: 2.0559x; 1.0084x over previous
"""Trainium2 Bass kernel for nn_ADLS_13022340842024 (moe_routing).

Data-parallel over batch across 8 NeuronCores (2048 samples/core).

Key algorithmic reductions (host-side, weight/index-only prep):
  * The gated domain-relation matrix Rg is a row-normalized diagonal =>
    h_prime = dom_emb[domain_id] exactly; all hierarchical routing (zeta,
    alpha) is a function of domain_id only -> per-sample LoRA scale rows
    gathered on host (sclb2, packed per superchunk).
  * Embedding lookup done on host: x shipped pre-transposed, chunk-packed
    bf16 -> one contiguous DMA per chunk, no gather.
  * Per-domain towers flattened to one [512,64] matmul + block-diagonal
    [64,8] second layer + one-hot select.

On-device per core:
  * 3-layer FCN backbone as feature-major bf16 matmuls; batch processed
    as 2 superchunks of 2x512 so each weight tile feeds two 512-wide
    matmuls back-to-back (longer PE chains, fewer stalls).
  * Weights arrive in column-block-major packed layouts: 4 DMAs per
    matrix, first o-blocks land in ~3us so L0 never waits.
  * LoRA: A-projections for the chunk pair run concurrently in PE column
    groups (tile_position), Bm folds run concurrently in PE row strips.
  * Tower first-layer k-chain interleaved into L2 epilogues to kill the
    end-of-kernel tail; one-hot domain select, PE ones-reduce.
"""
import numpy as np
import ml_dtypes
from contextlib import ExitStack

import concourse.bass as bass
import concourse.tile as tile
from concourse import bacc, mybir
from concourse import bass_utils

BF16 = ml_dtypes.bfloat16

B, F, V, ED = 16384, 32, 100000, 32
NCORES = 8
BL = B // NCORES                 # 2048 samples per core
IN, D0, D1, D2 = 1024, 2048, 1024, 512
D, E, L, R = 8, 8, 3, 4
ER = E * R                       # 32
CH = 512                         # batch chunk per core
NCHUNK = BL // CH                # 4
NSUP = NCHUNK // 2               # 2 superchunks of paired chunks
KT0, KT1, KT2 = IN // 128, D0 // 128, D1 // 128          # 8, 16, 8
OT0, OT1, OT2 = D0 // 128, D1 // 128, D2 // 128          # 16, 8, 4
EPS, EPS_LN, SCALING = 1e-8, 1e-5, 0.25

_CACHED_NC = None


def _build():
    nc = bacc.Bacc("TRN2", target_bir_lowering=False, debug=False)
    f32, f32r, bf16 = (mybir.dt.float32, mybir.dt.float32r, mybir.dt.bfloat16)

    xr_ext = nc.declare_dram_parameter("xr", [128, NCHUNK * KT0 * CH], bf16,
                                       isOutput=False)
    w0_ext = nc.declare_dram_parameter("w0c", [128, OT0 * KT0 * 128], bf16,
                                       isOutput=False)
    w1_ext = nc.declare_dram_parameter("w1c", [128, OT1 * KT1 * 128], bf16,
                                       isOutput=False)
    w2_ext = nc.declare_dram_parameter("w2c", [128, OT2 * KT2 * 128], bf16,
                                       isOutput=False)
    a0_ext = nc.declare_dram_parameter("a0p", [128, KT0 * ER], bf16, isOutput=False)
    a1_ext = nc.declare_dram_parameter("a1p", [128, KT1 * ER], bf16, isOutput=False)
    a2_ext = nc.declare_dram_parameter("a2p", [128, KT2 * ER], bf16, isOutput=False)
    bm0_ext = nc.declare_dram_parameter("bm0t", [2 * ER, D0], bf16, isOutput=False)
    bm1_ext = nc.declare_dram_parameter("bm1t", [2 * ER, D1], bf16, isOutput=False)
    bm2_ext = nc.declare_dram_parameter("bm2t", [2 * ER, D2], bf16, isOutput=False)
    sclb_ext = nc.declare_dram_parameter("sclb2", [2 * ER, 3 * NSUP * CH], bf16,
                                         isOutput=False)

    b0_ext = nc.declare_dram_parameter("b0p", [128, OT0], f32, isOutput=False)
    b1_ext = nc.declare_dram_parameter("b1p", [128, OT1], f32, isOutput=False)
    b2_ext = nc.declare_dram_parameter("b2p", [128, OT2], f32, isOutput=False)
    wt_ext = nc.declare_dram_parameter("wtc", [128, OT2 * 64], bf16, isOutput=False)
    bt1_ext = nc.declare_dram_parameter("bt1f2", [128, 1], f32, isOutput=False)
    m2_ext = nc.declare_dram_parameter("m2d", [128, 2 * D], f32r, isOutput=False)
    out_ext = nc.declare_dram_parameter("out", [2 * D, NSUP * CH], f32,
                                        isOutput=True)

    with tile.TileContext(nc) as tc, ExitStack() as ctx:
        wp = ctx.enter_context(tc.tile_pool(name="w", bufs=1))
        hp = ctx.enter_context(tc.tile_pool(name="h", bufs=1))
        sp = ctx.enter_context(tc.tile_pool(name="s", bufs=2))
        pp_mm = ctx.enter_context(tc.tile_pool(name="pmm", bufs=4, space="PSUM"))
        pp_lo = ctx.enter_context(tc.tile_pool(name="plo", bufs=1, space="PSUM"))
        pp_tw = ctx.enter_context(tc.tile_pool(name="ptw", bufs=1, space="PSUM"))

        def wsplit(ext, total_cols, nsplit, name, eng):
            """Load a packed weight as nsplit column-slice tiles."""
            tiles = []
            cols = total_cols // nsplit
            for i in range(nsplit):
                t = wp.tile([128, cols], bf16, tag=f"{name}{i}", name=f"{name}{i}")
                eng.dma_start(out=t[:, :], in_=ext[:, i * cols:(i + 1) * cols])
                tiles.append(t)
            return tiles

        # Weight col-splits alternate between the two HWDGE queues so the
        # early load runs at both queues' aggregate bandwidth; x/aux tensors
        # slot in by first-use time.
        def wtile(ext, total_cols, nsplit, i, name, eng):
            cols = total_cols // nsplit
            t = wp.tile([128, cols], bf16, tag=f"{name}{i}", name=f"{name}{i}")
            eng.dma_start(out=t[:, :], in_=ext[:, i * cols:(i + 1) * cols])
            return t

        def aux(ext, shape, dt, name, eng):
            t = wp.tile(shape, dt, tag=name, name=name)
            eng.dma_start(out=t[:, :], in_=ext[:, :])
            return t

        XC = KT0 * CH
        W0C, W1C, W2C = OT0 * KT0 * 128, OT1 * KT1 * 128, OT2 * KT2 * 128
        OB0, OB1, OB2 = OT0 // 8, OT1 // 4, OT2 // 4  # o-blocks per w tile

        xt = [None] * NCHUNK
        w0c, w1c, w2c = [None] * 8, [None] * 4, [None] * 4

        w0c[0] = wtile(w0_ext, W0C, 8, 0, "w0", nc.sync)
        # qAct stream, first-use order
        t = wp.tile([128, XC], bf16, tag="x0", name="x0")
        nc.scalar.dma_start(out=t[:, :], in_=xr_ext[:, 0:XC])
        xt[0] = t
        a0p = aux(a0_ext, [128, KT0 * ER], bf16, "a0p", nc.scalar)
        w0c[1] = wtile(w0_ext, W0C, 8, 1, "w0", nc.sync)
        sclb2 = aux(sclb_ext, [2 * ER, 3 * NSUP * CH], bf16, "sclb2", nc.scalar)
        bm0 = aux(bm0_ext, [2 * ER, D0], bf16, "bm0", nc.scalar)
        b0p = aux(b0_ext, [128, OT0], f32, "b0p", nc.scalar)
        w0c[2] = wtile(w0_ext, W0C, 8, 2, "w0", nc.sync)
        w0c[4] = wtile(w0_ext, W0C, 8, 4, "w0", nc.scalar)
        t = wp.tile([128, XC], bf16, tag="x1", name="x1")
        nc.scalar.dma_start(out=t[:, :], in_=xr_ext[:, XC:2 * XC])
        xt[1] = t
        w0c[3] = wtile(w0_ext, W0C, 8, 3, "w0", nc.sync)
        w0c[5] = wtile(w0_ext, W0C, 8, 5, "w0", nc.scalar)
        w0c[6] = wtile(w0_ext, W0C, 8, 6, "w0", nc.sync)
        w0c[7] = wtile(w0_ext, W0C, 8, 7, "w0", nc.scalar)
        w1c[0] = wtile(w1_ext, W1C, 4, 0, "w1", nc.sync)
        a1p = aux(a1_ext, [128, KT1 * ER], bf16, "a1p", nc.scalar)
        bm1 = aux(bm1_ext, [2 * ER, D1], bf16, "bm1", nc.scalar)
        b1p = aux(b1_ext, [128, OT1], f32, "b1p", nc.scalar)
        w1c[2] = wtile(w1_ext, W1C, 4, 2, "w1", nc.sync)
        w1c[1] = wtile(w1_ext, W1C, 4, 1, "w1", nc.scalar)
        w1c[3] = wtile(w1_ext, W1C, 4, 3, "w1", nc.scalar)
        w2c[0] = wtile(w2_ext, W2C, 4, 0, "w2", nc.sync)
        a2p = aux(a2_ext, [128, KT2 * ER], bf16, "a2p", nc.scalar)
        bm2 = aux(bm2_ext, [2 * ER, D2], bf16, "bm2", nc.scalar)
        b2p = aux(b2_ext, [128, OT2], f32, "b2p", nc.scalar)
        w2c[2] = wtile(w2_ext, W2C, 4, 2, "w2", nc.sync)
        w2c[1] = wtile(w2_ext, W2C, 4, 1, "w2", nc.scalar)
        w2c[3] = wtile(w2_ext, W2C, 4, 3, "w2", nc.scalar)
        wtc = wtile(wt_ext, OT2 * 64, 1, 0, "wt", nc.sync)
        bt1f2 = aux(bt1_ext, [128, 1], f32, "bt1f2", nc.sync)
        m2d = aux(m2_ext, [128, 2 * D], f32r, "m2d", nc.sync)
        t = wp.tile([128, XC], bf16, tag="x2", name="x2")
        nc.scalar.dma_start(out=t[:, :], in_=xr_ext[:, 2 * XC:3 * XC])
        xt[2] = t
        t = wp.tile([128, XC], bf16, tag="x3", name="x3")
        nc.scalar.dma_start(out=t[:, :], in_=xr_ext[:, 3 * XC:4 * XC])
        xt[3] = t

        relu = mybir.ActivationFunctionType.Relu

        def layer(rhs0, rhs1, kt, ot, obpt, wc, a_tile, bm_tile, bias_tile,
                  l_idx, sup, out0, out1, tw=None):
            """One FCN layer on the superchunk's chunk pair.

            rhs0/rhs1: fn(k) -> [128, CH] AP. Each weight tile feeds two
            matmuls; A-projection col-packed, Bm folds row-strip-packed.
            tw=(ps_twA, ps_twB) interleaves the tower k-chain per o-tile.
            """
            ps_t = pp_lo.tile([2 * ER, CH], f32, tag="lo")
            for k in range(kt):
                nc.tensor.matmul(out=ps_t[0:ER, :],
                                 lhsT=a_tile[:, k * ER:(k + 1) * ER],
                                 rhs=rhs0(k), start=(k == 0), stop=(k == kt - 1),
                                 tile_position=(0, 0))
                nc.tensor.matmul(out=ps_t[ER:2 * ER, :],
                                 lhsT=a_tile[:, k * ER:(k + 1) * ER],
                                 rhs=rhs1(k), start=(k == 0), stop=(k == kt - 1),
                                 tile_position=(0, ER))
            t2s = sp.tile([2 * ER, CH], bf16, tag="t2s")
            nc.vector.tensor_tensor(
                out=t2s[:, :], in0=ps_t[:, :],
                in1=sclb2[:, (l_idx * NSUP + sup) * CH:
                          (l_idx * NSUP + sup + 1) * CH],
                op=mybir.AluOpType.mult)
            for o in range(ot):
                wt_ = wc[o // obpt]
                off = (o % obpt) * kt * 128
                ps0 = pp_mm.tile([128, CH], f32, tag="mm")
                ps1 = pp_mm.tile([128, CH], f32, tag="mm")
                for k in range(kt):
                    lhsT = wt_[:, off + k * 128:off + (k + 1) * 128]
                    nc.tensor.matmul(out=ps0[:, :], lhsT=lhsT, rhs=rhs0(k),
                                     start=(k == 0), stop=False)
                    nc.tensor.matmul(out=ps1[:, :], lhsT=lhsT, rhs=rhs1(k),
                                     start=(k == 0), stop=False)
                nc.tensor.matmul(out=ps0[:, :],
                                 lhsT=bm_tile[0:ER, o * 128:(o + 1) * 128],
                                 rhs=t2s[0:ER, :], start=False, stop=True,
                                 tile_position=(0, 0))
                nc.tensor.matmul(out=ps1[:, :],
                                 lhsT=bm_tile[ER:2 * ER, o * 128:(o + 1) * 128],
                                 rhs=t2s[ER:2 * ER, :], start=False, stop=True,
                                 tile_position=(ER, 0))
                nc.scalar.activation(out=out0[o][:, :], in_=ps0[:, :],
                                     func=relu, bias=bias_tile[:, o:o + 1],
                                     scale=1.0)
                nc.scalar.activation(out=out1[o][:, :], in_=ps1[:, :],
                                     func=relu, bias=bias_tile[:, o:o + 1],
                                     scale=1.0)
                if tw is not None:
                    nc.tensor.matmul(out=tw[0:64, :],
                                     lhsT=wtc[:, o * 64:(o + 1) * 64],
                                     rhs=out0[o][:, :],
                                     start=(o == 0), stop=(o == ot - 1),
                                     tile_position=(0, 0))
                    nc.tensor.matmul(out=tw[64:128, :],
                                     lhsT=wtc[:, o * 64:(o + 1) * 64],
                                     rhs=out1[o][:, :],
                                     start=(o == 0), stop=(o == ot - 1),
                                     tile_position=(0, 64))

        def tower_tail(s, ps_tw):
            t1s2 = sp.tile([128, CH], f32r, tag="t1s2")
            nc.vector.tensor_scalar(out=t1s2[:, :], in0=ps_tw[:, :],
                                    scalar1=bt1f2[:, :], scalar2=0.0,
                                    op0=mybir.AluOpType.add,
                                    op1=mybir.AluOpType.max)
            ps_l = pp_tw.tile([2 * D, CH], f32, tag="twl")
            nc.tensor.matmul(out=ps_l[:, :], lhsT=m2d[:, :], rhs=t1s2[:, :],
                             start=True, stop=True)
            outc = sp.tile([2 * D, CH], f32, tag="oc")
            nc.vector.tensor_copy(out=outc[:, :], in_=ps_l[:, :])
            nc.sync.dma_start(out=out_ext[:, s * CH:(s + 1) * CH],
                              in_=outc[:, :])

        for s in range(NSUP):
            c0 = 2 * s
            rx0 = lambda k, _t=xt[c0]: _t[:, k * CH:(k + 1) * CH]
            rx1 = lambda k, _t=xt[c0 + 1]: _t[:, k * CH:(k + 1) * CH]
            h1a = [hp.tile([128, CH], bf16, name=f"h1a_{o}", tag=f"h1a_{o}")
                   for o in range(OT0)]
            h1b = [hp.tile([128, CH], bf16, name=f"h1b_{o}", tag=f"h1b_{o}")
                   for o in range(OT0)]
            layer(rx0, rx1, KT0, OT0, OB0, w0c, a0p, bm0, b0p, 0, s, h1a, h1b)
            rh1a = lambda k: h1a[k][:, :]
            rh1b = lambda k: h1b[k][:, :]
            h2a = [hp.tile([128, CH], bf16, name=f"h2a_{o}", tag=f"h2a_{o}")
                   for o in range(OT1)]
            h2b = [hp.tile([128, CH], bf16, name=f"h2b_{o}", tag=f"h2b_{o}")
                   for o in range(OT1)]
            layer(rh1a, rh1b, KT1, OT1, OB1, w1c, a1p, bm1, b1p, 1, s, h2a, h2b)
            rh2a = lambda k: h2a[k][:, :]
            rh2b = lambda k: h2b[k][:, :]
            h3a = [hp.tile([128, CH], bf16, name=f"h3a_{o}", tag=f"h3a_{o}")
                   for o in range(OT2)]
            h3b = [hp.tile([128, CH], bf16, name=f"h3b_{o}", tag=f"h3b_{o}")
                   for o in range(OT2)]
            ps_tw2 = pp_tw.tile([128, CH], f32, tag="tw2")
            layer(rh2a, rh2b, KT2, OT2, OB2, w2c, a2p, bm2, b2p, 2, s,
                  h3a, h3b, tw=ps_tw2)
            tower_tail(s, ps_tw2)

    nc.compile()
    return nc


def get_nc():
    global _CACHED_NC
    if _CACHED_NC is None:
        _CACHED_NC = _build()
    return _CACHED_NC


# ---------------- host-side math (exact fp32 mirror of the reference) -------

def _softplus(x):
    return np.logaddexp(0.0, x)


def _ln(x, g, b):
    m = x.mean(-1, keepdims=True)
    v = ((x - m) ** 2).mean(-1, keepdims=True)
    return g * (x - m) / np.sqrt(v + EPS_LN) + b


def _softmax(x):
    e = np.exp(x - x.max(-1, keepdims=True))
    return e / e.sum(-1, keepdims=True)


def _topk_sparse(p, k):
    idx = np.argsort(-p, axis=-1, kind="stable")[..., :k]
    mask = np.zeros_like(p)
    np.put_along_axis(mask, idx, 1.0, axis=-1)
    s = p * mask
    return s / np.maximum(s.sum(-1, keepdims=True), EPS)


def _routing_tables(dom_emb, layer_pos, gate_logits, Wi1, bi1, gi, bti, Wi2,
                    bi2, Wr1, br1, gr, btr, Wr2, br2):
    gate = _softplus(gate_logits.astype(np.float32))
    Rg = np.eye(D, dtype=np.float32) * gate
    Rg = Rg / np.maximum(Rg.sum(1, keepdims=True), EPS)
    hd = Rg @ dom_emb.astype(np.float32)                      # [D, 64]
    ri = np.concatenate([
        np.broadcast_to(hd[:, None, :], (D, L, hd.shape[-1])),
        np.broadcast_to(layer_pos[None].astype(np.float32), (D, L, layer_pos.shape[-1])),
    ], axis=-1)                                               # [D, L, 96]
    hi = np.maximum(_ln(ri @ Wi1.T + bi1, gi, bti), 0.0)
    scores = (hi @ Wi2.T + bi2)[..., 0]
    scores = scores - scores.max(-1, keepdims=True)
    phi = _softmax(scores)
    zeta = _topk_sparse(phi, min(2, L))                       # [D, L]
    hr = np.maximum(_ln(ri @ Wr1.T + br1, gr, btr), 0.0)
    alpha = _topk_sparse(_softmax(hr @ Wr2.T + br2), 2)       # [D, L, E]
    return zeta.astype(np.float32), alpha.astype(np.float32)


def _pack_w(W, kt, ot):
    """[out, in] -> [128, ot*kt*128] with (p, o, k, cc) layout (lhsT blocks)."""
    Wr = np.asarray(W, np.float32).reshape(ot, 128, kt, 128)   # [o, cc, k, p]
    return np.ascontiguousarray(
        Wr.transpose(3, 0, 2, 1).reshape(128, ot * kt * 128)).astype(BF16)


def _pack_a(A, kt):
    """[E, R, in] -> [128, kt*ER] with (p, k, r) layout."""
    Af = np.asarray(A, np.float32).transpose(2, 0, 1).reshape(-1, ER)  # [in, ER]
    Ar = Af.reshape(kt, 128, ER)
    return np.ascontiguousarray(
        Ar.transpose(1, 0, 2).reshape(128, kt * ER)).astype(BF16)


def kernel(field_idx, domain_id, emb_table, W0, b0, W1, b1, W2, b2,
           A0, Bm0, A1, Bm1, A2, Bm2, dom_emb, layer_pos, gate_logits,
           Wi1, bi1, gi, bti, Wi2, bi2, Wr1, br1, gr, btr, Wr2, br2,
           Wt1, bt1, Wt2, bt2):
    field_idx = np.asarray(field_idx)
    domain_id = np.asarray(domain_id)
    emb = np.asarray(emb_table, dtype=np.float32)

    zeta, alpha = _routing_tables(
        np.asarray(dom_emb), np.asarray(layer_pos), np.asarray(gate_logits),
        np.asarray(Wi1), np.asarray(bi1), np.asarray(gi), np.asarray(bti),
        np.asarray(Wi2), np.asarray(bi2), np.asarray(Wr1), np.asarray(br1),
        np.asarray(gr), np.asarray(btr), np.asarray(Wr2), np.asarray(br2))

    # per-layer LoRA scale tables packed [D, 3*E*R] (column block per layer)
    scl = np.zeros((D, 3 * ER), dtype=np.float32)
    for l in range(3):
        scl[:, l * ER:(l + 1) * ER] = (
            np.repeat(alpha[:, l, :], R, axis=1) * zeta[:, l, None] * SCALING)

    def prep_bm(Bm):
        bm = np.asarray(Bm, np.float32).transpose(0, 2, 1).reshape(ER, -1)
        return np.ascontiguousarray(np.tile(bm, (2, 1))).astype(BF16)

    shared = {
        "w0c": _pack_w(W0, KT0, OT0),
        "w1c": _pack_w(W1, KT1, OT1),
        "w2c": _pack_w(W2, KT2, OT2),
        "a0p": _pack_a(A0, KT0),
        "a1p": _pack_a(A1, KT1),
        "a2p": _pack_a(A2, KT2),
        "bm0t": prep_bm(Bm0), "bm1t": prep_bm(Bm1), "bm2t": prep_bm(Bm2),
        "b0p": np.ascontiguousarray(np.asarray(b0, np.float32).reshape(OT0, 128).T),
        "b1p": np.ascontiguousarray(np.asarray(b1, np.float32).reshape(OT1, 128).T),
        "b2p": np.ascontiguousarray(np.asarray(b2, np.float32).reshape(OT2, 128).T),
        "bt1f2": np.tile(np.asarray(bt1, np.float32).reshape(64, 1), (2, 1)),
    }
    # wtc: [512, 64] lhsT -> (p, k, cc) layout, k over 4 tiles of 128
    wtt = np.asarray(Wt1, np.float32).reshape(D * 8, D2).T       # [512, 64]
    wtr = wtt.reshape(OT2, 128, 64)                              # [k, p, cc]
    shared["wtc"] = np.ascontiguousarray(
        wtr.transpose(1, 0, 2).reshape(128, OT2 * 64)).astype(BF16)

    # M2[d*8+o, d'] = Wt2[d, 0, o] iff d == d'; block-diag for the chunk pair
    m2 = np.zeros((64, D), dtype=np.float32)
    wt2 = np.asarray(Wt2, np.float32)
    for d in range(D):
        m2[d * 8:(d + 1) * 8, d] = wt2[d, 0, :]
    m2d = np.zeros((128, 2 * D), dtype=np.float32)
    m2d[0:64, 0:D] = m2
    m2d[64:128, D:2 * D] = m2
    shared["m2d"] = m2d

    # host embedding lookup -> chunk-packed feature-major bf16
    x = emb[field_idx.astype(np.int64)].reshape(B, IN)
    xT = x.T.astype(BF16)                                        # [IN, B]

    in_maps = []
    for ci in range(NCORES):
        sl = slice(ci * BL, (ci + 1) * BL)
        dom = domain_id[sl].astype(np.int64)

        # xr: [128, c, k, j] packed
        xc = xT[:, sl].reshape(KT0, 128, NCHUNK, CH)             # [k, p, c, j]
        xr = np.ascontiguousarray(
            xc.transpose(1, 2, 0, 3).reshape(128, NCHUNK * KT0 * CH))
        # sclb2: [l*64 + ci*32 + r, s*CH + j]
        sr = scl[dom]                                            # [BL, 96]
        sclb2 = np.zeros((2 * ER, 3 * NSUP * CH), dtype=np.float32)
        for s in range(NSUP):
            for half in range(2):
                c = 2 * s + half
                blk = sr[c * CH:(c + 1) * CH]                    # [CH, 96]
                for l in range(3):
                    sclb2[half * ER:(half + 1) * ER,
                          (l * NSUP + s) * CH:(l * NSUP + s + 1) * CH] = \
                        blk[:, l * ER:(l + 1) * ER].T
        m = dict(shared)
        m.update({"xr": xr, "sclb2": sclb2.astype(BF16)})
        in_maps.append(m)

    nc = get_nc()
    res = bass_utils.run_bass_kernel_spmd(nc, in_maps, core_ids=list(range(NCORES)))
    bt2f = np.asarray(bt2, np.float32).reshape(D)
    out = np.empty(B, dtype=np.float32)
    for ci in range(NCORES):
        lg = np.asarray(res.results[ci]["out"], np.float32)  # [2D, NSUP*CH]
        dom = domain_id[ci * BL:(ci + 1) * BL].astype(np.int64)
        j = np.arange(BL)
        s, r = j // (2 * CH), j % (2 * CH)
        half, jj = r // CH, r % CH
        out[ci * BL:(ci + 1) * BL] = (
            lg[half * D + dom, s * CH + jj] + bt2f[dom])
    return out
